# revision 1
# baseline (speedup 1.0000x reference)
"""Tensor-parallel MiniGPT single-token decode step on 8 Trainium2 NeuronCores.

Sharding (per core i of 8):
  - attention: heads 2i, 2i+1 (head_dim 128 -> cols i*256:(i+1)*256 of E=2048);
    wq/wk/wv row-sharded, wo column-sharded, KV cache column-sharded by head.
  - MLP: w1 row-sharded (1024 rows/core), w2 column-sharded.
  - LM head: vocab-sharded (50257 padded to 8*6283=50264 rows).
  - Two 8KB AllReduces combine the wo- and w2- partial sums; logits are
    gathered on the host.

All weights are pre-laid-out on the host into [128, F] partition-major arrays
so every device DMA is one contiguous run per partition. All compute is fp32.

Matvec strategy: fp32 PE matmuls with a [128, 1] stationary run at only
~32 MACs/cycle, so the contraction is spread across three engines:
  - DVE / GPSIMD: acc[p, n] += W_tile[p, n] * x[k*128+p] via
    scalar_tensor_tensor, then one PE ones-vector matmul per 512-col chunk
    reduces across partitions (fp32 throughout).
  - PE: direct fp32 matmul chains (lhsT = x chunk) for a share of columns.
This keeps every engine's busy time below the DMA streaming time.
"""

import numpy as np

N_CORES = 8
E = 2048
HPC = 2  # heads per core
EPC = HPC * 128  # 256
T = 8192
VOCAB = 50257
VPC = 6283  # padded vocab rows per core (8 * 6283 = 50264)
SCALE = float(1.0 / np.sqrt(128.0))
EPS = 1e-5

_CACHE = {}
TRACE = False


def _build_nc():
    import concourse.bacc as bacc
    import concourse.mybir as mybir
    import concourse.tile as tile
    from concourse.masks import make_identity

    AF = mybir.ActivationFunctionType
    AX = mybir.AxisListType
    MUL = mybir.AluOpType.mult
    ADD = mybir.AluOpType.add
    dt = mybir.dt.float32

    nc = bacc.Bacc(
        "TRN2", target_bir_lowering=False, debug=False, num_devices=N_CORES
    )

    xe_wte = nc.declare_dram_parameter("xe_wte", [128, 16], dt, isOutput=False)
    xe_wpe = nc.declare_dram_parameter("xe_wpe", [128, 16], dt, isOutput=False)
    wqkv_r = nc.declare_dram_parameter("wqkv_r", [128, 16 * 768], dt, isOutput=False)
    keys_r = nc.declare_dram_parameter("keys_r", [128, 2 * 8192], dt, isOutput=False)
    vals_r = nc.declare_dram_parameter("vals_r", [128, 64 * 256], dt, isOutput=False)
    wo_r = nc.declare_dram_parameter("wo_r", [128, 2 * 2048], dt, isOutput=False)
    w1_r = nc.declare_dram_parameter("w1_r", [128, 16 * 1024], dt, isOutput=False)
    w2_r = nc.declare_dram_parameter("w2_r", [128, 8 * 2048], dt, isOutput=False)
    lm_r = nc.declare_dram_parameter("lm_r", [128, 16 * VPC], dt, isOutput=False)
    logits_out = nc.declare_dram_parameter("logits", [1, VPC], dt, isOutput=True)

    with tile.TileContext(nc) as tc:
        with (
            tc.tile_pool(name="const", bufs=1) as const,
            tc.tile_pool(name="small", bufs=1) as small,
            tc.tile_pool(name="stage", bufs=2) as stage,
            tc.tile_pool(name="ps", bufs=7, space="PSUM") as ps,
            tc.tile_pool(name="dram", bufs=1, space="DRAM") as dram,
            tc.tile_pool(name="stream", bufs=13) as stream,
            tc.tile_pool(name="acc", bufs=2) as accp,
        ):
            _snum = [0]

            def stile(label, width=2048):
                _snum[0] += 1
                return stream.tile(
                    [128, width], dt, tag="s", name=f"s{_snum[0]}_{label}"
                )

            ones_col = const.tile([128, 1], dt)
            nc.vector.memset(ones_col[:], 1.0)
            ones_row = const.tile([1, 128], dt)
            nc.vector.memset(ones_row[:], 1.0)
            ident = const.tile([128, 128], dt)
            make_identity(nc, ident[:])
            eps_c = const.tile([1, 1], dt)
            nc.vector.memset(eps_c[:], EPS)
            wscr = const.tile([128, 512], dt)
            nc.vector.memset(wscr[:], 0.25)

            # Warm up the collectives path off the critical path: the first
            # collective in a NEFF pays a ~13us init cost.
            warm_in = dram.tile([1, 16], dt, tag="warm_in")
            warm_out = dram.tile([N_CORES, 16], dt, tag="warm_out")
            warm_sb = stage.tile([1, 16], dt, tag="warm", bufs=1)
            nc.vector.memset(warm_sb[:], 0.0)
            nc.gpsimd.dma_start(warm_in[:], warm_sb[:])
            nc.gpsimd.collective_compute(
                "AllGather",
                mybir.AluOpType.bypass,
                replica_groups=[list(range(N_CORES))],
                ins=[warm_in.opt()],
                outs=[warm_out.opt()],
            )
            warm_back = stage.tile([1, 16], dt, tag="warmb", bufs=1)
            nc.gpsimd.dma_start(warm_back[:], warm_out[0:1, :])

            def rms(xt, name, extra=None):
                """x * rsqrt(mean(x^2) + eps) for x in [128, 16] column layout.

                extra: optional [1, 16]-shaped nuisance row added with weight 0
                (keeps the warmup collective's output alive against DCE).
                """
                sq = small.tile([128, 16], dt, tag=f"sq_{name}")
                ssum = small.tile([128, 1], dt, tag=f"ss_{name}")
                nc.scalar.activation(sq[:], xt[:], AF.Square, accum_out=ssum[:])
                tot = ps.tile([1, 1], dt, tag="b")
                nc.tensor.matmul(tot[:], ssum[:], ones_col[:], start=True, stop=True)
                std = small.tile([1, 1], dt, tag=f"std_{name}")
                nc.scalar.activation(
                    std[:], tot[:], AF.Sqrt, bias=eps_c[:], scale=1.0 / float(E)
                )
                inv = small.tile([1, 1], dt, tag=f"inv_{name}")
                nc.vector.reciprocal(inv[:], std[:])
                invb_ps = ps.tile([128, 1], dt, tag="b")
                nc.tensor.matmul(
                    invb_ps[:], ones_row[:], inv[:], start=True, stop=True
                )
                xn = small.tile([128, 16], dt, tag=f"xn_{name}")
                nc.vector.tensor_scalar_mul(xn[:], xt[:], invb_ps[:])
                return xn

            # ---- embedding + double rms ----
            xw = stage.tile([128, 16], dt, tag="xw")
            nc.scalar.dma_start(xw[:], xe_wte[:])
            xp = stage.tile([128, 16], dt, tag="xp")
            nc.scalar.dma_start(xp[:], xe_wpe[:])
            x0 = small.tile([128, 16], dt, tag="x0")
            nc.vector.tensor_add(x0[:], xw[:], xp[:])
            x1 = rms(x0, "n1")  # residual input
            x2 = rms(x1, "n2")

            # ---- qkv projection: [1, 768] row (q 0:256 | k 256:512 | v 512:768)
            for k in range(16):
                wt = stile("qkv", 768)
                nc.sync.dma_start(wt[:], wqkv_r[:, k * 768 : (k + 1) * 768])
                if k == 0:
                    ps_q = ps.tile([1, 512], dt, tag="b", name="ps_q")
                    ps_v = ps.tile([1, 256], dt, tag="b", name="ps_v")
                nc.tensor.matmul(
                    ps_q[:], x2[:, k : k + 1], wt[:, 0:512],
                    start=(k == 0), stop=(k == 15),
                )
                nc.tensor.matmul(
                    ps_v[:], x2[:, k : k + 1], wt[:, 512:768],
                    start=(k == 0), stop=(k == 15),
                )
            qkv_row = small.tile([1, 768], dt, tag="qkv")
            nc.scalar.mul(qkv_row[:, 0:256], ps_q[:, 0:256], SCALE)
            nc.scalar.copy(qkv_row[:, 256:512], ps_q[:, 256:512])
            nc.scalar.copy(qkv_row[:, 512:768], ps_v[:])

            # ---- transpose q,k to column layout: qkT[:, 0:2]=q heads, 2:4=k heads
            st4 = stage.tile([4, 128], dt, tag="st4")
            nc.scalar.dma_start(st4[:], qkv_row[:, 0:512])
            qkT_ps = ps.tile([128, 4], dt, tag="b")
            nc.tensor.transpose(qkT_ps[:], st4[:], ident[0:4, 0:4])
            qkT = small.tile([128, 4], dt, tag="qkT")
            nc.vector.tensor_copy(qkT[:], qkT_ps[:])

            # ---- attention scores: row-layout QK matmuls, PSUM drained by
            # DVE/ACT copies, reshaped via SBUF DMA + PE transpose; exp on the
            # [128, 16] transposed tiles. wT_h[p, c] = exp(att_h[c*128 + p]).
            wTs = []
            esp = small.tile([128, 16], dt, tag="esp")  # per-partition exp sums
            for h in range(HPC):
                wTs.append(small.tile([128, 64], dt, tag=f"wT{h}", name=f"wT{h}"))
            for h in range(HPC):
                for j in range(4):
                    kt = stile("key")
                    nc.sync.dma_start(
                        kt[:],
                        keys_r[:, h * 8192 + j * 2048 : h * 8192 + (j + 1) * 2048],
                    )
                    att_row = small.tile(
                        [1, 2048], dt, tag="attrow", name=f"attrow{h}_{j}", bufs=3
                    )
                    for n in range(4):
                        pa = ps.tile([1, 512], dt, tag="b")
                        nc.tensor.matmul(
                            pa[:], qkT[:, h : h + 1],
                            kt[:, n * 512 : (n + 1) * 512],
                            start=True, stop=True,
                        )
                        if n != 3:
                            nc.vector.tensor_copy(
                                att_row[:, n * 512 : (n + 1) * 512], pa[:]
                            )
                        else:
                            nc.scalar.copy(att_row[:, n * 512 : (n + 1) * 512], pa[:])
                    wst = stage.tile([16, 128], dt, tag="wst", name=f"wst{h}_{j}", bufs=4)
                    nc.scalar.dma_start(wst[:], att_row[:])
                    wps = ps.tile([128, 16], dt, tag="b", name=f"wps{h}_{j}")
                    nc.tensor.transpose(wps[:], wst[:], ident[0:16, 0:16])
                    nc.scalar.activation(
                        wTs[h][:, j * 16 : (j + 1) * 16], wps[:], AF.Exp,
                        accum_out=esp[:, h * 4 + j : h * 4 + j + 1],
                    )
                    # keep the PE's HAM clock warm across the kt DMA wait
                    wm = ps.tile([1, 512], dt, tag="wm", bufs=1, name=f"wm{h}_{j}")
                    nc.tensor.matmul(wm[:], ones_col[:], wscr[:], start=True, stop=True)
                    nc.tensor.matmul(wm[:], ones_col[:], wscr[:], start=True, stop=True)

            # current-token score per head: exp(q_h . k_h) (SCALE folded into q)
            e_last = small.tile([1, 2], dt, tag="elast")
            for h in range(HPC):
                pal = ps.tile([1, 1], dt, tag="b")
                nc.tensor.matmul(
                    pal[:], qkT[:, h : h + 1], qkT[:, 2 + h : 3 + h],
                    start=True, stop=True,
                )
                nc.scalar.activation(e_last[:, h : h + 1], pal[:], AF.Exp)

            # softmax denominators: cross-partition sum of esp + e_last
            dps = ps.tile([1, 8], dt, tag="b")
            nc.tensor.matmul(dps[:], ones_col[:], esp[:, 0:8], start=True, stop=True)
            dtmp = small.tile([1, 2], dt, tag="dtmp")
            for h in range(HPC):
                nc.vector.reduce_sum(
                    dtmp[:, h : h + 1], dps[:, h * 4 : (h + 1) * 4], axis=AX.X
                )
            nc.vector.tensor_add(dtmp[:], dtmp[:], e_last[:])
            dinv = small.tile([1, 2], dt, tag="dinv")
            nc.vector.reciprocal(dinv[:], dtmp[:])

            # ---- PV: x_attn_h = sum_t w[t] * V[t, :] (unnormalized) on DVE
            acc_pv = [
                accp.tile([128, 128], dt, tag=f"accpv{h}", name=f"acc_pv{h}")
                for h in range(HPC)
            ]
            for tt in range(8):
                vt = stile("val")
                nc.sync.dma_start(vt[:], vals_r[:, tt * 2048 : (tt + 1) * 2048])
                for j in range(8):
                    c = tt * 8 + j
                    for h in range(HPC):
                        vsl = vt[:, j * 256 + h * 128 : j * 256 + (h + 1) * 128]
                        if c == 0:
                            nc.vector.tensor_scalar_mul(
                                acc_pv[h][:], vsl, wTs[h][:, 0:1]
                            )
                        else:
                            nc.vector.scalar_tensor_tensor(
                                acc_pv[h][:], vsl, wTs[h][:, c : c + 1], acc_pv[h][:],
                                op0=MUL, op1=ADD,
                            )
            pv_ps = []
            for h in range(HPC):
                p = ps.tile([1, 128], dt, tag="b", name=f"pv_ps{h}")
                nc.tensor.matmul(p[:], ones_col[:], acc_pv[h][:], start=True, stop=True)
                pv_ps.append(p)

            # combine with current-token value, then normalize by the softmax sum
            xa_row = small.tile([1, 256], dt, tag="xa")
            for h in range(HPC):
                sl = slice(h * 128, (h + 1) * 128)
                nc.vector.tensor_scalar_mul(
                    xa_row[:, sl],
                    qkv_row[:, 512 + h * 128 : 512 + (h + 1) * 128],
                    e_last[:, h : h + 1],
                )
                nc.vector.tensor_add(xa_row[:, sl], xa_row[:, sl], pv_ps[h][:])
                nc.vector.tensor_scalar_mul(xa_row[:, sl], xa_row[:, sl], dinv[:, h : h + 1])

            # ---- transpose x_attn to column layout [128, 2] ----
            st2 = stage.tile([2, 128], dt, tag="st2")
            nc.scalar.dma_start(st2[:], xa_row[:])
            xaT_ps = ps.tile([128, 2], dt, tag="b")
            nc.tensor.transpose(xaT_ps[:], st2[:], ident[0:2, 0:2])
            xaT = small.tile([128, 2], dt, tag="xaT")
            nc.vector.tensor_copy(xaT[:], xaT_ps[:])

            # ---- wo partial: [1, 2048]; DVE cols 0:1024, PE chains 1024:2048
            ar1_in = small.tile([1, 2048], dt, tag="arin", name="ar1_in")
            acc_wo = accp.tile([128, 1024], dt, tag="acc", name="acc_wo")
            wo_pe = [ps.tile([1, 512], dt, tag="b", name=f"wope{n}") for n in range(2)]
            for k in range(2):
                wot = stile("wo")
                nc.sync.dma_start(wot[:], wo_r[:, k * 2048 : (k + 1) * 2048])
                if k == 0:
                    nc.vector.tensor_scalar_mul(acc_wo[:], wot[:, 0:1024], xaT[:, 0:1])
                else:
                    nc.vector.scalar_tensor_tensor(
                        acc_wo[:], wot[:, 0:1024], xaT[:, 1:2], acc_wo[:],
                        op0=MUL, op1=ADD,
                    )
                for n in range(2):
                    nc.tensor.matmul(
                        wo_pe[n][:], xaT[:, k : k + 1],
                        wot[:, 1024 + n * 512 : 1024 + (n + 1) * 512],
                        start=(k == 0), stop=(k == 1),
                    )
            for n in range(2):
                po = ps.tile([1, 512], dt, tag="b")
                nc.tensor.matmul(
                    po[:], ones_col[:], acc_wo[:, n * 512 : (n + 1) * 512],
                    start=True, stop=True,
                )
                nc.vector.tensor_copy(ar1_in[:, n * 512 : (n + 1) * 512], po[:])
            for n in range(2):
                nc.vector.tensor_copy(
                    ar1_in[:, 1024 + n * 512 : 1024 + (n + 1) * 512], wo_pe[n][:]
                )

            def all_reduce(row_sb, name):
                """Sum a [1, 2048] partial across cores; returns PSUM [128, 16].

                AllGather + on-core rank reduction: 16 K=8 matmuls against a
                ones vector sum the 8 gathered rows and transpose into the
                [128, 16] column layout.
                """
                in_d = dram.tile([1, 2048], dt, tag=f"{name}_in")
                out_d = dram.tile([N_CORES, 2048], dt, tag=f"{name}_out")
                nc.scalar.dma_start(in_d[:], row_sb[:])
                nc.gpsimd.collective_compute(
                    "AllGather",
                    mybir.AluOpType.bypass,
                    replica_groups=[list(range(N_CORES))],
                    ins=[in_d.opt()],
                    outs=[out_d.opt()],
                )
                ag_sb = stage.tile(
                    [N_CORES, 2048], dt, tag="ag", name=f"ag_{name}", bufs=1
                )
                nc.scalar.dma_start(ag_sb[:], out_d[:])
                x_ps = ps.tile([128, 16], dt, tag="b", name=f"xps_{name}")
                for c in range(16):
                    nc.tensor.matmul(
                        x_ps[:, c : c + 1],
                        ag_sb[:, c * 128 : (c + 1) * 128],
                        ones_col[0:N_CORES, :],
                        start=True, stop=True,
                    )
                return x_ps

            # keep the warmup-collective result alive: ar1_in[0, :16] += 0*warm
            nc.vector.scalar_tensor_tensor(
                ar1_in[0:1, 0:16], warm_back[:], 0.0, ar1_in[0:1, 0:16],
                op0=MUL, op1=ADD,
            )
            x3_ps = all_reduce(ar1_in, "ar1")
            x3 = small.tile([128, 16], dt, tag="x3")
            nc.vector.tensor_add(x3[:], x3_ps[:], x1[:])  # + residual

            # ---- MLP1: h = relu(w1 @ x4) on DVE ----
            x4 = rms(x3, "n3")
            acc_h1 = accp.tile([128, 1024], dt, tag="acc", name="acc_h1")
            for a in range(8):
                w1t = stile("w1")
                nc.sync.dma_start(w1t[:], w1_r[:, a * 2048 : (a + 1) * 2048])
                for b in range(2):
                    k = a * 2 + b
                    wsl = w1t[:, b * 1024 : (b + 1) * 1024]
                    if k == 0:
                        nc.vector.tensor_scalar_mul(acc_h1[:], wsl, x4[:, 0:1])
                    else:
                        nc.vector.scalar_tensor_tensor(
                            acc_h1[:], wsl, x4[:, k : k + 1], acc_h1[:],
                            op0=MUL, op1=ADD,
                        )
            h_row = small.tile([1, 1024], dt, tag="hrow")
            for n in range(2):
                phn = ps.tile([1, 512], dt, tag="b", name=f"ph{n}")
                nc.tensor.matmul(
                    phn[:], ones_col[:], acc_h1[:, n * 512 : (n + 1) * 512],
                    start=True, stop=True,
                )
                nc.scalar.activation(h_row[:, n * 512 : (n + 1) * 512], phn[:], AF.Relu)

            st8 = stage.tile([8, 128], dt, tag="st8")
            nc.scalar.dma_start(st8[:], h_row[:])
            hT_ps = ps.tile([128, 8], dt, tag="b")
            nc.tensor.transpose(hT_ps[:], st8[:], ident[0:8, 0:8])
            hT = small.tile([128, 8], dt, tag="hT")
            nc.vector.tensor_copy(hT[:], hT_ps[:])

            # ---- MLP2: DVE cols 0:1024 of each k-block, PE chains 1024:2048
            ar2_in = small.tile([1, 2048], dt, tag="arin", name="ar2_in")
            acc_m2 = accp.tile([128, 1024], dt, tag="acc", name="acc_m2")
            pm_pe = [ps.tile([1, 512], dt, tag="b", name=f"pmpe{n}") for n in range(2)]
            for k in range(8):
                w2t = stile("w2")
                nc.sync.dma_start(w2t[:], w2_r[:, k * 2048 : (k + 1) * 2048])
                if k == 0:
                    nc.vector.tensor_scalar_mul(acc_m2[:], w2t[:, 0:1024], hT[:, 0:1])
                else:
                    nc.vector.scalar_tensor_tensor(
                        acc_m2[:], w2t[:, 0:1024], hT[:, k : k + 1], acc_m2[:],
                        op0=MUL, op1=ADD,
                    )
                for n in range(2):
                    nc.tensor.matmul(
                        pm_pe[n][:], hT[:, k : k + 1],
                        w2t[:, 1024 + n * 512 : 1024 + (n + 1) * 512],
                        start=(k == 0), stop=(k == 7),
                    )
            for n in range(2):
                pm = ps.tile([1, 512], dt, tag="b", name=f"pm{n}")
                nc.tensor.matmul(
                    pm[:], ones_col[:], acc_m2[:, n * 512 : (n + 1) * 512],
                    start=True, stop=True,
                )
                nc.vector.tensor_copy(ar2_in[:, n * 512 : (n + 1) * 512], pm[:])
            for n in range(2):
                nc.vector.tensor_copy(
                    ar2_in[:, 1024 + n * 512 : 1024 + (n + 1) * 512], pm_pe[n][:]
                )

            x5_ps = all_reduce(ar2_in, "ar2")
            x5 = small.tile([128, 16], dt, tag="x5")
            nc.vector.tensor_add(x5[:], x5_ps[:], x3[:])  # + residual (x3)

            # ---- LM head over the vocab shard, two passes of 512-col chunks.
            # Per pass/k: lt_a [128, 2048] + lt_b [128, w-2048]; DVE accumulates
            # lt_a cols 0:1536, PE runs direct chains for the remaining columns.
            passes = [(0, 3584), (3584, VPC)]
            DVE_W = 1536
            for lo, hi in passes:
                width = hi - lo
                w_b = width - 2048
                acc_lm = accp.tile([128, DVE_W], dt, tag="acclm", name=f"acc_lm{lo}")
                pe_w = width - DVE_W
                npe = (pe_w + 511) // 512
                pe_ps = [
                    ps.tile(
                        [1, min(512, pe_w - 512 * i)], dt, tag="b", name=f"pe{lo}_{i}"
                    )
                    for i in range(npe)
                ]
                for k in range(16):
                    lt_a = stile("lma")
                    nc.sync.dma_start(
                        lt_a[:], lm_r[:, k * VPC + lo : k * VPC + lo + 2048]
                    )
                    lt_b = stile("lmb")
                    nc.sync.dma_start(
                        lt_b[:, 0:w_b], lm_r[:, k * VPC + lo + 2048 : k * VPC + hi]
                    )
                    if k == 0:
                        nc.vector.tensor_scalar_mul(
                            acc_lm[:], lt_a[:, 0:DVE_W], x5[:, 0:1]
                        )
                    else:
                        nc.vector.scalar_tensor_tensor(
                            acc_lm[:], lt_a[:, 0:DVE_W], x5[:, k : k + 1], acc_lm[:],
                            op0=MUL, op1=ADD,
                        )
                    for i in range(npe):
                        cw = pe_ps[i].shape[1]
                        coff = DVE_W + i * 512
                        if coff + cw <= 2048:
                            rhs = lt_a[:, coff : coff + cw]
                        else:
                            rhs = lt_b[:, coff - 2048 : coff - 2048 + cw]
                        nc.tensor.matmul(
                            pe_ps[i][:], x5[:, k : k + 1], rhs,
                            start=(k == 0), stop=(k == 15),
                        )
                for n in range(3):
                    pl = ps.tile([1, 512], dt, tag="b", name=f"pla{lo}_{n}")
                    nc.tensor.matmul(
                        pl[:], ones_col[:], acc_lm[:, n * 512 : (n + 1) * 512],
                        start=True, stop=True,
                    )
                    lr = small.tile([1, 512], dt, tag="lrow", name=f"lr{lo}_{n}", bufs=3)
                    nc.vector.tensor_copy(lr[:], pl[:])
                    nc.scalar.dma_start(
                        logits_out[:, lo + n * 512 : lo + (n + 1) * 512], lr[:]
                    )
                for i in range(npe):
                    cw = pe_ps[i].shape[1]
                    lr = small.tile(
                        [1, 512], dt, tag="lrow", name=f"lrp{lo}_{i}", bufs=3
                    )
                    nc.vector.tensor_copy(lr[:, 0:cw], pe_ps[i][:])
                    nc.scalar.dma_start(
                        logits_out[:, lo + DVE_W + i * 512 : lo + DVE_W + i * 512 + cw],
                        lr[:, 0:cw],
                    )

    nc.finalize()
    return nc


def _col16(v):
    """[2048] vector -> [128, 16] column-major layout (e = c*128 + p at [p, c])."""
    return np.ascontiguousarray(v.reshape(16, 128).T)


def _part_major(mT, nblk, blk_rows, width):
    """[nblk*blk_rows, width] -> [blk_rows, nblk*width] partition-major."""
    return np.ascontiguousarray(
        mT.reshape(nblk, blk_rows, width).transpose(1, 0, 2).reshape(blk_rows, nblk * width)
    )


def _prep_in_maps(token_id, pos_id, keys, values, wte, wpe, wq, wk, wv, wo, w1, w2, lm_w):
    f32 = lambda a: np.asarray(a, dtype=np.float32)
    keys, values = f32(keys), f32(values)
    wq, wk, wv, wo, w1, w2, lm_w = map(f32, (wq, wk, wv, wo, w1, w2, lm_w))
    xe_wte = _col16(f32(wte[token_id]))
    xe_wpe = _col16(f32(wpe[pos_id]))
    lm_pad = np.zeros((N_CORES * VPC, E), np.float32)
    lm_pad[:VOCAB] = lm_w

    in_maps = []
    for i in range(N_CORES):
        hs = slice(i * EPC, (i + 1) * EPC)
        wqkv = np.concatenate([wq[hs], wk[hs], wv[hs]], axis=0)  # [768, E]
        in_maps.append(
            {
                "xe_wte": xe_wte,
                "xe_wpe": xe_wpe,
                "wqkv_r": _part_major(np.ascontiguousarray(wqkv.T), 16, 128, 768),
                "keys_r": _part_major(np.ascontiguousarray(keys[:, hs].T), 2, 128, 8192),
                "vals_r": _part_major(values[:, hs], 64, 128, EPC),
                "wo_r": _part_major(np.ascontiguousarray(wo[:, hs].T), 2, 128, E),
                "w1_r": _part_major(
                    np.ascontiguousarray(w1[i * 1024 : (i + 1) * 1024].T), 16, 128, 1024
                ),
                "w2_r": _part_major(
                    np.ascontiguousarray(w2[:, i * 1024 : (i + 1) * 1024].T), 8, 128, E
                ),
                "lm_r": _part_major(
                    np.ascontiguousarray(lm_pad[i * VPC : (i + 1) * VPC].T), 16, 128, VPC
                ),
            }
        )
    return in_maps


def kernel(**inputs) -> np.ndarray:
    from concourse.bass_utils import run_bass_kernel_spmd

    token_id = int(inputs["token_id"])
    pos_id = int(inputs["pos_id"])
    in_maps = _prep_in_maps(
        token_id,
        pos_id,
        inputs["keys"],
        inputs["values"],
        inputs["wte"],
        inputs["wpe"],
        inputs["wq"],
        inputs["wk"],
        inputs["wv"],
        inputs["wo"],
        inputs["w1"],
        inputs["w2"],
        inputs["lm_w"],
    )
    if "nc" not in _CACHE:
        _CACHE["nc"] = _build_nc()
    nc = _CACHE["nc"]
    res = run_bass_kernel_spmd(
        nc,
        in_maps,
        core_ids=list(range(N_CORES)),
        trace=TRACE,
        trace_cores=[0] if TRACE else None,
    )
    _CACHE["last_result"] = res
    logits = np.concatenate([r["logits"][0] for r in res.results])[:VOCAB]
    return np.ascontiguousarray(logits.astype(np.float32))



# revision 2
# speedup vs baseline: 1.7061x; 1.7061x over previous
"""Tensor-parallel MiniGPT single-token decode step on 8 Trainium2 NeuronCores.

Sharding (per core i of 8):
  - attention: heads 2i, 2i+1 (head_dim 128 -> cols i*256:(i+1)*256 of E=2048);
    wq/wk/wv row-sharded, wo column-sharded, KV cache column-sharded by head.
  - MLP: w1 row-sharded (1024 rows/core), w2 column-sharded.
  - LM head: vocab-sharded (50257 padded to 8*6283=50264 rows).
  - Two 8KB AllReduces combine the wo- and w2- partial sums; logits are
    gathered on the host.

All weights are cast to bf16 on the host and laid out as [128, F]
partition-major arrays, so every device DMA is one contiguous ~1MB run per
partition at half the fp32 byte count. Activations stay fp32; PE matmuls take
bf16 operands and accumulate fp32 in PSUM.

Matvec strategy: PE does nearly everything (bf16 matmuls run 4x faster than
fp32). x-chunks are the [128, 1] stationary operand (LDWEIGHTS P=1 is ~free),
weight tiles stream as the moving operand in N=512 chunks with PSUM
accumulation across the 16 k-blocks. Attention scores are computed directly in
column layout (key-block [128d, 128t] stationary, q [128, 1] moving), which
removes the row->column transpose round-trip; exp runs on ACT straight from
PSUM with accum_out providing softmax denominators. The DVE takes a fixed
1024-column share of the LM head via scalar_tensor_tensor so no single engine
exceeds the DMA streaming time.
"""

import numpy as np
import ml_dtypes

N_CORES = 8
E = 2048
HPC = 2  # heads per core
EPC = HPC * 128  # 256
T = 8192
VOCAB = 50257
VPC = 6283  # padded vocab rows per core (8 * 6283 = 50264)
SCALE = float(1.0 / np.sqrt(128.0))
EPS = 1e-5

_CACHE = {}
TRACE = False


def _build_nc():
    import concourse.bacc as bacc
    import concourse.mybir as mybir
    import concourse.tile as tile

    AF = mybir.ActivationFunctionType
    AX = mybir.AxisListType
    MUL = mybir.AluOpType.mult
    ADD = mybir.AluOpType.add
    dt = mybir.dt.float32
    bf = mybir.dt.bfloat16

    nc = bacc.Bacc(
        "TRN2", target_bir_lowering=False, debug=False, num_devices=N_CORES
    )

    xe_wte = nc.declare_dram_parameter("xe_wte", [128, 16], dt, isOutput=False)
    xe_wpe = nc.declare_dram_parameter("xe_wpe", [128, 16], dt, isOutput=False)
    wqkv_r = nc.declare_dram_parameter("wqkv_r", [128, 16 * 768], bf, isOutput=False)
    keys_r = nc.declare_dram_parameter("keys_r", [128, 2 * 8192], bf, isOutput=False)
    vals_r = nc.declare_dram_parameter("vals_r", [128, 64 * 256], bf, isOutput=False)
    wo_r = nc.declare_dram_parameter("wo_r", [128, 2 * 2048], bf, isOutput=False)
    w1_r = nc.declare_dram_parameter("w1_r", [128, 16 * 1024], bf, isOutput=False)
    w2_r = nc.declare_dram_parameter("w2_r", [128, 8 * 2048], bf, isOutput=False)
    lm_r = nc.declare_dram_parameter("lm_r", [128, 16 * VPC], bf, isOutput=False)
    logits_out = nc.declare_dram_parameter("logits", [1, VPC], dt, isOutput=True)

    with tile.TileContext(nc) as tc:
        with (
            tc.tile_pool(name="const", bufs=1) as const,
            tc.tile_pool(name="small", bufs=1) as small,
            tc.tile_pool(name="stage", bufs=2) as stage,
            tc.tile_pool(name="ps", bufs=7, space="PSUM") as ps,
            tc.tile_pool(name="dram", bufs=1, space="DRAM") as dram,
            tc.tile_pool(name="stream", bufs=13) as stream,
            tc.tile_pool(name="acc", bufs=2) as accp,
        ):
            _snum = [0]

            def stile(label, width=4096):
                _snum[0] += 1
                return stream.tile(
                    [128, width], bf, tag="s", name=f"s{_snum[0]}_{label}"
                )

            ones_col = const.tile([128, 1], dt)
            nc.vector.memset(ones_col[:], 1.0)
            ones_row = const.tile([1, 128], dt)
            nc.vector.memset(ones_row[:], 1.0)
            from concourse.masks import make_identity

            ident = const.tile([128, 128], dt)
            make_identity(nc, ident[:])
            eps_c = const.tile([1, 1], dt)
            nc.vector.memset(eps_c[:], EPS)

            # Warm up the collectives path off the critical path: the first
            # collective in a NEFF pays a ~13us init cost.
            warm_in = dram.tile([1, 16], dt, tag="warm_in")
            warm_out = dram.tile([N_CORES, 16], dt, tag="warm_out")
            warm_sb = stage.tile([1, 16], dt, tag="warm", bufs=1)
            nc.vector.memset(warm_sb[:], 0.0)
            nc.gpsimd.dma_start(warm_in[:], warm_sb[:])
            nc.gpsimd.collective_compute(
                "AllGather",
                mybir.AluOpType.bypass,
                replica_groups=[list(range(N_CORES))],
                ins=[warm_in.opt()],
                outs=[warm_out.opt()],
            )
            warm_back = stage.tile([1, 16], dt, tag="warmb", bufs=1)
            nc.gpsimd.dma_start(warm_back[:], warm_out[0:1, :])

            def rms(xt, name):
                """x * rsqrt(mean(x^2) + eps) for x in [128, 16] column layout."""
                sq = small.tile([128, 16], dt, tag=f"sq_{name}")
                ssum = small.tile([128, 1], dt, tag=f"ss_{name}")
                nc.scalar.activation(sq[:], xt[:], AF.Square, accum_out=ssum[:])
                tot = ps.tile([1, 1], dt, tag="b")
                nc.tensor.matmul(tot[:], ssum[:], ones_col[:], start=True, stop=True)
                std = small.tile([1, 1], dt, tag=f"std_{name}")
                nc.scalar.activation(
                    std[:], tot[:], AF.Sqrt, bias=eps_c[:], scale=1.0 / float(E)
                )
                inv = small.tile([1, 1], dt, tag=f"inv_{name}")
                nc.vector.reciprocal(inv[:], std[:])
                invb_ps = ps.tile([128, 1], dt, tag="b")
                nc.tensor.matmul(
                    invb_ps[:], ones_row[:], inv[:], start=True, stop=True
                )
                xn = small.tile([128, 16], dt, tag=f"xn_{name}")
                nc.vector.tensor_scalar_mul(xn[:], xt[:], invb_ps[:])
                return xn

            def to_bf(xt, name, w=16):
                xb = small.tile([128, w], bf, tag=f"bf_{name}")
                nc.vector.tensor_copy(xb[:], xt[:])
                return xb

            # ---- embedding + double rms ----
            xw = stage.tile([128, 16], dt, tag="xw")
            nc.scalar.dma_start(xw[:], xe_wte[:])
            xp = stage.tile([128, 16], dt, tag="xp")
            nc.scalar.dma_start(xp[:], xe_wpe[:])
            x0 = small.tile([128, 16], dt, tag="x0")
            nc.vector.tensor_add(x0[:], xw[:], xp[:])
            x1 = rms(x0, "n1")  # residual input
            x2 = rms(x1, "n2")
            x2b = to_bf(x2, "x2")

            # ---- qkv projection: [1, 768] row (q 0:256 | k 256:512 | v 512:768)
            # SCALE is folded into wq on the host.
            for t in range(4):
                wt = stile("qkv", 3072)
                nc.sync.dma_start(wt[:], wqkv_r[:, t * 3072 : (t + 1) * 3072])
                if t == 0:
                    ps_q = ps.tile([1, 512], dt, tag="b", name="ps_q")
                    ps_v = ps.tile([1, 256], dt, tag="b", name="ps_v")
                for b in range(4):
                    k = 4 * t + b
                    nc.tensor.matmul(
                        ps_q[:], x2b[:, k : k + 1], wt[:, b * 768 : b * 768 + 512],
                        start=(k == 0), stop=(k == 15),
                    )
                    nc.tensor.matmul(
                        ps_v[:], x2b[:, k : k + 1], wt[:, b * 768 + 512 : b * 768 + 768],
                        start=(k == 0), stop=(k == 15),
                    )
            qkv_row = small.tile([1, 768], dt, tag="qkv")
            nc.scalar.copy(qkv_row[:, 0:512], ps_q[:])
            nc.scalar.copy(qkv_row[:, 512:768], ps_v[:])

            # ---- transpose q,k to column layout: qkT[:, 0:2]=q heads, 2:4=k heads
            st4 = stage.tile([4, 128], dt, tag="st4")
            nc.scalar.dma_start(st4[:], qkv_row[:, 0:512])
            qkT_ps = ps.tile([128, 4], dt, tag="b")
            nc.tensor.transpose(qkT_ps[:], st4[:], ident[0:4, 0:4])
            qkTb = small.tile([128, 4], bf, tag="qkTb")
            nc.vector.tensor_copy(qkTb[:], qkT_ps[:])

            # ---- attention scores in column layout: per head h and 128-row
            # t-block c, matmul(out[128,1], lhsT=K_block[128d,128t], rhs=q[128,1])
            # gives scores for t in [c*128, (c+1)*128) on partitions. ACT exp
            # straight from PSUM, accum_out accumulates softmax denominators.
            wTs = []
            esp = small.tile([128, 4], dt, tag="esp")  # per-partition exp sums
            for h in range(HPC):
                wTs.append(small.tile([128, 64], bf, tag=f"wT{h}", name=f"wT{h}"))
            for h in range(HPC):
                for j in range(2):
                    kt = stile("key")
                    nc.sync.dma_start(
                        kt[:],
                        keys_r[:, h * 8192 + j * 4096 : h * 8192 + (j + 1) * 4096],
                    )
                    qk_ps = ps.tile([128, 32], dt, tag="b", name=f"qk{h}_{j}")
                    for c in range(32):
                        nc.tensor.matmul(
                            qk_ps[:, c : c + 1],
                            kt[:, c * 128 : (c + 1) * 128],
                            qkTb[:, h : h + 1],
                            start=True, stop=True,
                        )
                    nc.scalar.activation(
                        wTs[h][:, j * 32 : (j + 1) * 32], qk_ps[:], AF.Exp,
                        accum_out=esp[:, h * 2 + j : h * 2 + j + 1],
                    )

            # current-token score per head: exp(q_h . k_h) (SCALE folded into q)
            e_last = small.tile([1, 2], dt, tag="elast")
            for h in range(HPC):
                pal = ps.tile([1, 1], dt, tag="b")
                nc.tensor.matmul(
                    pal[:], qkTb[:, h : h + 1], qkTb[:, 2 + h : 3 + h],
                    start=True, stop=True,
                )
                nc.scalar.activation(e_last[:, h : h + 1], pal[:], AF.Exp)

            # softmax denominators: cross-partition sum of esp + e_last
            dps = ps.tile([1, 4], dt, tag="b")
            nc.tensor.matmul(dps[:], ones_col[:], esp[:], start=True, stop=True)
            dtmp = small.tile([1, 2], dt, tag="dtmp")
            for h in range(HPC):
                nc.vector.reduce_sum(
                    dtmp[:, h : h + 1], dps[:, h * 2 : (h + 1) * 2], axis=AX.X
                )
            nc.vector.tensor_add(dtmp[:], dtmp[:], e_last[:])
            dinv = small.tile([1, 2], dt, tag="dinv")
            nc.vector.reciprocal(dinv[:], dtmp[:])

            # ---- PV on PE: x_attn_h = sum_t w[t] * V[t, :] (unnormalized).
            # Per t-block c: matmul(out[1,128], lhsT=wT[:, c], rhs=V_block)
            # accumulating in PSUM over all 64 blocks.
            pv_ps = [
                ps.tile([1, 128], dt, tag="b", name=f"pv_ps{h}") for h in range(HPC)
            ]
            for tt in range(4):
                vt = stile("val")
                nc.sync.dma_start(vt[:], vals_r[:, tt * 4096 : (tt + 1) * 4096])
                for b in range(16):
                    c = tt * 16 + b
                    for h in range(HPC):
                        nc.tensor.matmul(
                            pv_ps[h][:],
                            wTs[h][:, c : c + 1],
                            vt[:, b * 256 + h * 128 : b * 256 + (h + 1) * 128],
                            start=(c == 0), stop=(c == 63),
                        )

            # combine with current-token value, then normalize by the softmax sum
            xa_row = small.tile([1, 256], dt, tag="xa")
            for h in range(HPC):
                sl = slice(h * 128, (h + 1) * 128)
                nc.vector.tensor_scalar_mul(
                    xa_row[:, sl],
                    qkv_row[:, 512 + h * 128 : 512 + (h + 1) * 128],
                    e_last[:, h : h + 1],
                )
                nc.vector.tensor_add(xa_row[:, sl], xa_row[:, sl], pv_ps[h][:])
                nc.vector.tensor_scalar_mul(xa_row[:, sl], xa_row[:, sl], dinv[:, h : h + 1])

            # ---- transpose x_attn to column layout [128, 2] ----
            st2 = stage.tile([2, 128], dt, tag="st2")
            nc.scalar.dma_start(st2[:], xa_row[:])
            xaT_ps = ps.tile([128, 2], dt, tag="b")
            nc.tensor.transpose(xaT_ps[:], st2[:], ident[0:2, 0:2])
            xaTb = small.tile([128, 2], bf, tag="xaTb")
            nc.vector.tensor_copy(xaTb[:], xaT_ps[:])

            # ---- wo partial: [1, 2048] via 4 PE chains ----
            ar1_in = small.tile([1, 2048], dt, tag="arin", name="ar1_in")
            wo_pe = [ps.tile([1, 512], dt, tag="b", name=f"wope{n}") for n in range(4)]
            wot = stile("wo")
            nc.sync.dma_start(wot[:], wo_r[:])
            for k in range(2):
                for n in range(4):
                    nc.tensor.matmul(
                        wo_pe[n][:], xaTb[:, k : k + 1],
                        wot[:, k * 2048 + n * 512 : k * 2048 + (n + 1) * 512],
                        start=(k == 0), stop=(k == 1),
                    )
            for n in range(4):
                nc.vector.tensor_copy(
                    ar1_in[:, n * 512 : (n + 1) * 512], wo_pe[n][:]
                )

            def all_reduce(row_sb, name):
                """Sum a [1, 2048] partial across cores; returns PSUM [128, 16].

                AllGather + on-core rank reduction: 16 K=8 matmuls against a
                ones vector sum the 8 gathered rows and transpose into the
                [128, 16] column layout.
                """
                in_d = dram.tile([1, 2048], dt, tag=f"{name}_in")
                out_d = dram.tile([N_CORES, 2048], dt, tag=f"{name}_out")
                nc.scalar.dma_start(in_d[:], row_sb[:])
                nc.gpsimd.collective_compute(
                    "AllGather",
                    mybir.AluOpType.bypass,
                    replica_groups=[list(range(N_CORES))],
                    ins=[in_d.opt()],
                    outs=[out_d.opt()],
                )
                ag_sb = stage.tile(
                    [N_CORES, 2048], dt, tag="ag", name=f"ag_{name}", bufs=1
                )
                nc.scalar.dma_start(ag_sb[:], out_d[:])
                x_ps = ps.tile([128, 16], dt, tag="b", name=f"xps_{name}")
                for c in range(16):
                    nc.tensor.matmul(
                        x_ps[:, c : c + 1],
                        ag_sb[:, c * 128 : (c + 1) * 128],
                        ones_col[0:N_CORES, :],
                        start=True, stop=True,
                    )
                return x_ps

            # keep the warmup-collective result alive: ar1_in[0, :16] += 0*warm
            nc.vector.scalar_tensor_tensor(
                ar1_in[0:1, 0:16], warm_back[:], 0.0, ar1_in[0:1, 0:16],
                op0=MUL, op1=ADD,
            )
            x3_ps = all_reduce(ar1_in, "ar1")
            x3 = small.tile([128, 16], dt, tag="x3")
            nc.vector.tensor_add(x3[:], x3_ps[:], x1[:])  # + residual

            # ---- MLP1: h = relu(w1 @ x4) via 2 PE chains ----
            x4 = rms(x3, "n3")
            x4b = to_bf(x4, "x4")
            ph = [ps.tile([1, 512], dt, tag="b", name=f"ph{n}") for n in range(2)]
            for a in range(4):
                w1t = stile("w1")
                nc.sync.dma_start(w1t[:], w1_r[:, a * 4096 : (a + 1) * 4096])
                for b in range(4):
                    k = a * 4 + b
                    for n in range(2):
                        nc.tensor.matmul(
                            ph[n][:], x4b[:, k : k + 1],
                            w1t[:, b * 1024 + n * 512 : b * 1024 + (n + 1) * 512],
                            start=(k == 0), stop=(k == 15),
                        )
            h_row = small.tile([1, 1024], dt, tag="hrow")
            for n in range(2):
                nc.scalar.activation(h_row[:, n * 512 : (n + 1) * 512], ph[n][:], AF.Relu)

            st8 = stage.tile([8, 128], dt, tag="st8")
            nc.scalar.dma_start(st8[:], h_row[:])
            hT_ps = ps.tile([128, 8], dt, tag="b")
            nc.tensor.transpose(hT_ps[:], st8[:], ident[0:8, 0:8])
            hTb = small.tile([128, 8], bf, tag="hTb")
            nc.vector.tensor_copy(hTb[:], hT_ps[:])

            # ---- MLP2: [1, 2048] partial via 4 PE chains ----
            ar2_in = small.tile([1, 2048], dt, tag="arin", name="ar2_in")
            pm = [ps.tile([1, 512], dt, tag="b", name=f"pm{n}") for n in range(4)]
            for a in range(4):
                w2t = stile("w2")
                nc.sync.dma_start(w2t[:], w2_r[:, a * 4096 : (a + 1) * 4096])
                for b in range(2):
                    k = a * 2 + b
                    for n in range(4):
                        nc.tensor.matmul(
                            pm[n][:], hTb[:, k : k + 1],
                            w2t[:, b * 2048 + n * 512 : b * 2048 + (n + 1) * 512],
                            start=(k == 0), stop=(k == 7),
                        )
            for n in range(4):
                nc.vector.tensor_copy(ar2_in[:, n * 512 : (n + 1) * 512], pm[n][:])

            x5_ps = all_reduce(ar2_in, "ar2")
            x5 = small.tile([128, 16], dt, tag="x5")
            nc.vector.tensor_add(x5[:], x5_ps[:], x3[:])  # + residual (x3)
            x5b = to_bf(x5, "x5")

            # ---- LM head over the vocab shard, two passes. Per pass/k one DMA
            # of [128, width]; DVE accumulates cols 0:1024 via stt, PE chains
            # the rest in N<=512 chunks.
            passes = [(0, 3584), (3584, VPC)]
            DVE_W = 1024
            for lo, hi in passes:
                width = hi - lo
                acc_lm = accp.tile([128, DVE_W], dt, tag="acclm", name=f"acc_lm{lo}")
                pe_w = width - DVE_W
                npe = (pe_w + 511) // 512
                pe_ps = [
                    ps.tile(
                        [1, min(512, pe_w - 512 * i)], dt, tag="b", name=f"pe{lo}_{i}"
                    )
                    for i in range(npe)
                ]
                for k in range(16):
                    lt = stile("lm", 3584)
                    nc.sync.dma_start(
                        lt[:, 0:width], lm_r[:, k * VPC + lo : k * VPC + hi]
                    )
                    if k == 0:
                        nc.vector.tensor_scalar_mul(
                            acc_lm[:], lt[:, 0:DVE_W], x5[:, 0:1]
                        )
                    else:
                        nc.vector.scalar_tensor_tensor(
                            acc_lm[:], lt[:, 0:DVE_W], x5[:, k : k + 1], acc_lm[:],
                            op0=MUL, op1=ADD,
                        )
                    for i in range(npe):
                        cw = pe_ps[i].shape[1]
                        nc.tensor.matmul(
                            pe_ps[i][:], x5b[:, k : k + 1],
                            lt[:, DVE_W + i * 512 : DVE_W + i * 512 + cw],
                            start=(k == 0), stop=(k == 15),
                        )
                for n in range(2):
                    pl = ps.tile([1, 512], dt, tag="b", name=f"pla{lo}_{n}")
                    nc.tensor.matmul(
                        pl[:], ones_col[:], acc_lm[:, n * 512 : (n + 1) * 512],
                        start=True, stop=True,
                    )
                    lr = small.tile([1, 512], dt, tag="lrow", name=f"lr{lo}_{n}", bufs=3)
                    nc.vector.tensor_copy(lr[:], pl[:])
                    nc.scalar.dma_start(
                        logits_out[:, lo + n * 512 : lo + (n + 1) * 512], lr[:]
                    )
                for i in range(npe):
                    cw = pe_ps[i].shape[1]
                    lr = small.tile(
                        [1, 512], dt, tag="lrow", name=f"lrp{lo}_{i}", bufs=3
                    )
                    nc.vector.tensor_copy(lr[:, 0:cw], pe_ps[i][:])
                    nc.scalar.dma_start(
                        logits_out[:, lo + DVE_W + i * 512 : lo + DVE_W + i * 512 + cw],
                        lr[:, 0:cw],
                    )

    nc.finalize()
    return nc


def _col16(v):
    """[2048] vector -> [128, 16] column-major layout (e = c*128 + p at [p, c])."""
    return np.ascontiguousarray(v.reshape(16, 128).T)


def _part_major(mT, nblk, blk_rows, width):
    """[nblk*blk_rows, width] -> [blk_rows, nblk*width] partition-major, bf16."""
    out = mT.reshape(nblk, blk_rows, width).transpose(1, 0, 2).reshape(
        blk_rows, nblk * width
    )
    return np.ascontiguousarray(out.astype(ml_dtypes.bfloat16))


def _prep_in_maps(token_id, pos_id, keys, values, wte, wpe, wq, wk, wv, wo, w1, w2, lm_w):
    f32 = lambda a: np.asarray(a, dtype=np.float32)
    keys, values = f32(keys), f32(values)
    wq, wk, wv, wo, w1, w2, lm_w = map(f32, (wq, wk, wv, wo, w1, w2, lm_w))
    wq = wq * np.float32(SCALE)  # fold attention scale into q
    xe_wte = _col16(f32(wte[token_id]))
    xe_wpe = _col16(f32(wpe[pos_id]))
    lm_pad = np.zeros((N_CORES * VPC, E), np.float32)
    lm_pad[:VOCAB] = lm_w

    in_maps = []
    for i in range(N_CORES):
        hs = slice(i * EPC, (i + 1) * EPC)
        wqkv = np.concatenate([wq[hs], wk[hs], wv[hs]], axis=0)  # [768, E]
        in_maps.append(
            {
                "xe_wte": xe_wte,
                "xe_wpe": xe_wpe,
                "wqkv_r": _part_major(np.ascontiguousarray(wqkv.T), 16, 128, 768),
                "keys_r": _part_major(np.ascontiguousarray(keys[:, hs].T), 2, 128, 8192),
                "vals_r": _part_major(values[:, hs], 64, 128, EPC),
                "wo_r": _part_major(np.ascontiguousarray(wo[:, hs].T), 2, 128, E),
                "w1_r": _part_major(
                    np.ascontiguousarray(w1[i * 1024 : (i + 1) * 1024].T), 16, 128, 1024
                ),
                "w2_r": _part_major(
                    np.ascontiguousarray(w2[:, i * 1024 : (i + 1) * 1024].T), 8, 128, E
                ),
                "lm_r": _part_major(
                    np.ascontiguousarray(lm_pad[i * VPC : (i + 1) * VPC].T), 16, 128, VPC
                ),
            }
        )
    return in_maps


def kernel(**inputs) -> np.ndarray:
    from concourse.bass_utils import run_bass_kernel_spmd

    token_id = int(inputs["token_id"])
    pos_id = int(inputs["pos_id"])
    in_maps = _prep_in_maps(
        token_id,
        pos_id,
        inputs["keys"],
        inputs["values"],
        inputs["wte"],
        inputs["wpe"],
        inputs["wq"],
        inputs["wk"],
        inputs["wv"],
        inputs["wo"],
        inputs["w1"],
        inputs["w2"],
        inputs["lm_w"],
    )
    if "nc" not in _CACHE:
        _CACHE["nc"] = _build_nc()
    nc = _CACHE["nc"]
    res = run_bass_kernel_spmd(
        nc,
        in_maps,
        core_ids=list(range(N_CORES)),
        trace=TRACE,
        trace_cores=[0] if TRACE else None,
    )
    _CACHE["last_result"] = res
    logits = np.concatenate([r["logits"][0] for r in res.results])[:VOCAB]
    return np.ascontiguousarray(logits.astype(np.float32))


# revision 8
# speedup vs baseline: 1.8312x; 1.0733x over previous
"""Tensor-parallel MiniGPT single-token decode step on 8 Trainium2 NeuronCores.

Sharding (per core i of 8):
  - attention: heads 2i, 2i+1 (head_dim 128 -> cols i*256:(i+1)*256 of E=2048);
    wq/wk/wv row-sharded, wo column-sharded, KV cache column-sharded by head.
  - MLP: w1 row-sharded (1024 rows/core), w2 column-sharded.
  - LM head: vocab-sharded (50257 padded to 8*6283=50264 rows).
  - Two 8KB AllReduces combine the wo- and w2- partial sums; logits are
    gathered on the host.

All weights are cast to bf16 on the host and laid out as [128, F]
partition-major arrays, so every device DMA is one contiguous ~1MB run per
partition at half the fp32 byte count. Activations stay fp32; PE matmuls take
bf16 operands and accumulate fp32 in PSUM.

Matvec strategy: PE does nearly everything (bf16 matmuls run 4x faster than
fp32). x-chunks are the [128, 1] stationary operand (LDWEIGHTS P=1 is ~free),
weight tiles stream as the moving operand in N=512 chunks with PSUM
accumulation across the 16 k-blocks. Attention scores are computed directly in
column layout (key-block [128d, 128t] stationary, q [128, 1] moving), which
removes the row->column transpose round-trip; exp runs on ACT straight from
PSUM with accum_out providing softmax denominators. The DVE takes a fixed
1024-column share of the LM head via scalar_tensor_tensor so no single engine
exceeds the DMA streaming time.
"""

import numpy as np
import ml_dtypes

N_CORES = 8
E = 2048
HPC = 2  # heads per core
EPC = HPC * 128  # 256
T = 8192
VOCAB = 50257
VPC = 6283  # padded vocab rows per core (8 * 6283 = 50264)
SCALE = float(1.0 / np.sqrt(128.0))
EPS = 1e-5

_CACHE = {}
TRACE = False


def _build_nc():
    import concourse.bacc as bacc
    import concourse.mybir as mybir
    import concourse.tile as tile

    AF = mybir.ActivationFunctionType
    AX = mybir.AxisListType
    MUL = mybir.AluOpType.mult
    ADD = mybir.AluOpType.add
    dt = mybir.dt.float32
    bf = mybir.dt.bfloat16

    nc = bacc.Bacc(
        "TRN2", target_bir_lowering=False, debug=False, num_devices=N_CORES
    )

    xe_wte = nc.declare_dram_parameter("xe_wte", [128, 16], dt, isOutput=False)
    xe_wpe = nc.declare_dram_parameter("xe_wpe", [128, 16], dt, isOutput=False)
    wqkv_r = nc.declare_dram_parameter("wqkv_r", [128, 16 * 768], bf, isOutput=False)
    keys_r = nc.declare_dram_parameter("keys_r", [128, 2 * 8192], bf, isOutput=False)
    vals_r = nc.declare_dram_parameter("vals_r", [128, 64 * 256], bf, isOutput=False)
    wo_r = nc.declare_dram_parameter("wo_r", [128, 2 * 2048], bf, isOutput=False)
    w1_r = nc.declare_dram_parameter("w1_r", [128, 16 * 1024], bf, isOutput=False)
    w2_r = nc.declare_dram_parameter("w2_r", [128, 8 * 2048], bf, isOutput=False)
    lm_r = nc.declare_dram_parameter("lm_r", [128, 16 * VPC], bf, isOutput=False)
    logits_out = nc.declare_dram_parameter("logits", [1, VPC], dt, isOutput=True)

    with tile.TileContext(nc) as tc:
        with (
            tc.tile_pool(name="const", bufs=1) as const,
            tc.tile_pool(name="small", bufs=1) as small,
            tc.tile_pool(name="stage", bufs=2) as stage,
            tc.tile_pool(name="ps", bufs=7, space="PSUM") as ps,
            tc.tile_pool(name="dram", bufs=1, space="DRAM") as dram,
            tc.tile_pool(name="stream", bufs=20) as stream,
            tc.tile_pool(name="acc", bufs=2) as accp,
        ):
            _snum = [0]

            def stile(label, width=4096):
                _snum[0] += 1
                return stream.tile(
                    [128, width], bf, tag="s", name=f"s{_snum[0]}_{label}"
                )

            # Warm up the collectives path first: CC mesh init (~67us) starts
            # at the first collective trigger, so dispatch it as early as
            # possible. The result is consumed (x0) at the output tail only.
            warm_in = dram.tile([1, 16], dt, tag="warm_in")
            warm_out = dram.tile([N_CORES, 16], dt, tag="warm_out")
            warm_sb = stage.tile([1, 16], dt, tag="warm", bufs=1)
            nc.vector.memset(warm_sb[:], 0.0)
            nc.gpsimd.dma_start(warm_in[:], warm_sb[:])
            nc.gpsimd.collective_compute(
                "AllGather",
                mybir.AluOpType.bypass,
                replica_groups=[list(range(N_CORES))],
                ins=[warm_in.opt()],
                outs=[warm_out.opt()],
            )
            warm_back = stage.tile([1, 16], dt, tag="warmb", bufs=1)
            nc.gpsimd.dma_start(warm_back[:], warm_out[0:1, :])

            ones_col = const.tile([128, 1], dt)
            nc.vector.memset(ones_col[:], 1.0)
            ones_row = const.tile([1, 128], dt)
            nc.vector.memset(ones_row[:], 1.0)
            from concourse.masks import make_identity

            ident = const.tile([128, 128], dt)
            make_identity(nc, ident[:])
            eps_c = const.tile([1, 1], dt)
            nc.vector.memset(eps_c[:], EPS)

            def rms(xt, name):
                """x * rsqrt(mean(x^2) + eps) for x in [128, 16] column layout."""
                sq = small.tile([128, 16], dt, tag=f"sq_{name}")
                ssum = small.tile([128, 1], dt, tag=f"ss_{name}")
                nc.scalar.activation(sq[:], xt[:], AF.Square, accum_out=ssum[:])
                tot = ps.tile([1, 1], dt, tag="b")
                nc.tensor.matmul(tot[:], ssum[:], ones_col[:], start=True, stop=True)
                std = small.tile([1, 1], dt, tag=f"std_{name}")
                nc.scalar.activation(
                    std[:], tot[:], AF.Sqrt, bias=eps_c[:], scale=1.0 / float(E)
                )
                inv = small.tile([1, 1], dt, tag=f"inv_{name}")
                nc.vector.reciprocal(inv[:], std[:])
                invb_ps = ps.tile([128, 1], dt, tag="b")
                nc.tensor.matmul(
                    invb_ps[:], ones_row[:], inv[:], start=True, stop=True
                )
                xn = small.tile([128, 16], dt, tag=f"xn_{name}")
                nc.vector.tensor_scalar_mul(xn[:], xt[:], invb_ps[:])
                return xn

            def to_bf(xt, name, w=16):
                xb = small.tile([128, w], bf, tag=f"bf_{name}")
                nc.vector.tensor_copy(xb[:], xt[:])
                return xb

            # ---- embedding + double rms ----
            xw = stage.tile([128, 16], dt, tag="xw")
            nc.scalar.dma_start(xw[:], xe_wte[:])
            xp = stage.tile([128, 16], dt, tag="xp")
            nc.scalar.dma_start(xp[:], xe_wpe[:])
            x0 = small.tile([128, 16], dt, tag="x0")
            nc.vector.tensor_add(x0[:], xw[:], xp[:])
            x1 = rms(x0, "n1")  # residual input
            x2 = rms(x1, "n2")
            x2b = to_bf(x2, "x2")

            # ---- qkv projection: [1, 768] row (q 0:256 | k 256:512 | v 512:768)
            # SCALE is folded into wq on the host.
            for t in range(4):
                wt = stile("qkv", 3072)
                nc.sync.dma_start(wt[:], wqkv_r[:, t * 3072 : (t + 1) * 3072])
                if t == 0:
                    ps_q = ps.tile([1, 512], dt, tag="b", name="ps_q")
                    ps_v = ps.tile([1, 256], dt, tag="b", name="ps_v")
                for b in range(4):
                    k = 4 * t + b
                    nc.tensor.matmul(
                        ps_q[:], x2b[:, k : k + 1], wt[:, b * 768 : b * 768 + 512],
                        start=(k == 0), stop=(k == 15),
                    )
                    nc.tensor.matmul(
                        ps_v[:], x2b[:, k : k + 1], wt[:, b * 768 + 512 : b * 768 + 768],
                        start=(k == 0), stop=(k == 15),
                    )
            qkv_row = small.tile([1, 768], dt, tag="qkv")
            nc.scalar.copy(qkv_row[:, 0:512], ps_q[:])
            nc.scalar.copy(qkv_row[:, 512:768], ps_v[:])

            # ---- transpose q,k to column layout: qkT[:, 0:2]=q heads, 2:4=k heads
            st4 = stage.tile([4, 128], dt, tag="st4")
            nc.scalar.dma_start(st4[:], qkv_row[:, 0:512])
            qkT_ps = ps.tile([128, 4], dt, tag="b")
            nc.tensor.transpose(qkT_ps[:], st4[:], ident[0:4, 0:4])
            qkTb = small.tile([128, 4], bf, tag="qkTb")
            nc.vector.tensor_copy(qkTb[:], qkT_ps[:])

            # ---- attention scores in column layout: per head h and 128-row
            # t-block c, matmul(out[128,1], lhsT=K_block[128d,128t], rhs=q[128,1])
            # gives scores for t in [c*128, (c+1)*128) on partitions. ACT exp
            # straight from PSUM, accum_out accumulates softmax denominators.
            wTs = []
            esp = small.tile([128, 4], dt, tag="esp")  # per-partition exp sums
            for h in range(HPC):
                wTs.append(small.tile([128, 64], bf, tag=f"wT{h}", name=f"wT{h}"))
            for h in range(HPC):
                for j in range(2):
                    kt = stile("key")
                    nc.sync.dma_start(
                        kt[:],
                        keys_r[:, h * 8192 + j * 4096 : h * 8192 + (j + 1) * 4096],
                    )
                    qk_ps = ps.tile([128, 32], dt, tag="b", name=f"qk{h}_{j}")
                    for c in range(32):
                        nc.tensor.matmul(
                            qk_ps[:, c : c + 1],
                            kt[:, c * 128 : (c + 1) * 128],
                            qkTb[:, h : h + 1],
                            start=True, stop=True,
                        )
                    nc.scalar.activation(
                        wTs[h][:, j * 32 : (j + 1) * 32], qk_ps[:], AF.Exp,
                        accum_out=esp[:, h * 2 + j : h * 2 + j + 1],
                    )

            # current-token score per head: exp(q_h . k_h) (SCALE folded into q)
            e_last = small.tile([1, 2], dt, tag="elast")
            for h in range(HPC):
                pal = ps.tile([1, 1], dt, tag="b")
                nc.tensor.matmul(
                    pal[:], qkTb[:, h : h + 1], qkTb[:, 2 + h : 3 + h],
                    start=True, stop=True,
                )
                nc.scalar.activation(e_last[:, h : h + 1], pal[:], AF.Exp)

            # softmax denominators: cross-partition sum of esp + e_last
            dps = ps.tile([1, 4], dt, tag="b")
            nc.tensor.matmul(dps[:], ones_col[:], esp[:], start=True, stop=True)
            dtmp = small.tile([1, 2], dt, tag="dtmp")
            for h in range(HPC):
                nc.vector.reduce_sum(
                    dtmp[:, h : h + 1], dps[:, h * 2 : (h + 1) * 2], axis=AX.X
                )
            nc.vector.tensor_add(dtmp[:], dtmp[:], e_last[:])
            dinv = small.tile([1, 2], dt, tag="dinv")
            nc.vector.reciprocal(dinv[:], dtmp[:])

            # ---- PV on PE: x_attn_h = sum_t w[t] * V[t, :] (unnormalized).
            # Per t-block c: matmul(out[1,128], lhsT=wT[:, c], rhs=V_block)
            # accumulating in PSUM over all 64 blocks.
            pv_ps = [
                ps.tile([1, 128], dt, tag="b", name=f"pv_ps{h}") for h in range(HPC)
            ]
            for tt in range(4):
                vt = stile("val")
                nc.sync.dma_start(vt[:], vals_r[:, tt * 4096 : (tt + 1) * 4096])
                for b in range(16):
                    c = tt * 16 + b
                    for h in range(HPC):
                        nc.tensor.matmul(
                            pv_ps[h][:],
                            wTs[h][:, c : c + 1],
                            vt[:, b * 256 + h * 128 : b * 256 + (h + 1) * 128],
                            start=(c == 0), stop=(c == 63),
                        )

            # combine with current-token value, then normalize by the softmax sum
            xa_row = small.tile([1, 256], dt, tag="xa")
            for h in range(HPC):
                sl = slice(h * 128, (h + 1) * 128)
                nc.vector.tensor_scalar_mul(
                    xa_row[:, sl],
                    qkv_row[:, 512 + h * 128 : 512 + (h + 1) * 128],
                    e_last[:, h : h + 1],
                )
                nc.vector.tensor_add(xa_row[:, sl], xa_row[:, sl], pv_ps[h][:])
                nc.vector.tensor_scalar_mul(xa_row[:, sl], xa_row[:, sl], dinv[:, h : h + 1])

            # ---- transpose x_attn to column layout [128, 2] ----
            st2 = stage.tile([2, 128], dt, tag="st2")
            nc.scalar.dma_start(st2[:], xa_row[:])
            xaT_ps = ps.tile([128, 2], dt, tag="b")
            nc.tensor.transpose(xaT_ps[:], st2[:], ident[0:2, 0:2])
            xaTb = small.tile([128, 2], bf, tag="xaTb")
            nc.vector.tensor_copy(xaTb[:], xaT_ps[:])

            # ---- wo partial: [1, 2048] via 4 PE chains ----
            ar1_in = small.tile([1, 2048], dt, tag="arin", name="ar1_in")
            wo_pe = [ps.tile([1, 512], dt, tag="b", name=f"wope{n}") for n in range(4)]
            wot = stile("wo")
            nc.sync.dma_start(wot[:], wo_r[:])
            for k in range(2):
                for n in range(4):
                    nc.tensor.matmul(
                        wo_pe[n][:], xaTb[:, k : k + 1],
                        wot[:, k * 2048 + n * 512 : k * 2048 + (n + 1) * 512],
                        start=(k == 0), stop=(k == 1),
                    )
            for n in range(4):
                nc.vector.tensor_copy(
                    ar1_in[:, n * 512 : (n + 1) * 512], wo_pe[n][:]
                )

            def all_reduce(row_sb, name):
                """Sum a [1, 2048] partial across cores; returns PSUM [128, 16].

                CCE AllReduce in DRAM, then one reshaping readback DMA and a
                PE transpose into the [128, 16] column layout.
                """
                in_d = dram.tile([1, 2048], dt, tag=f"{name}_in")
                out_d = dram.tile([1, 2048], dt, tag=f"{name}_out")
                nc.scalar.dma_start(in_d[:], row_sb[:])
                nc.gpsimd.collective_compute(
                    "AllReduce",
                    ADD,
                    replica_groups=[list(range(N_CORES))],
                    ins=[in_d.opt()],
                    outs=[out_d.opt()],
                )
                st16 = stage.tile(
                    [16, 128], dt, tag="st16", name=f"st16_{name}", bufs=1
                )
                nc.scalar.dma_start(st16[:], out_d[:])
                x_ps = ps.tile([128, 16], dt, tag="b", name=f"xps_{name}")
                nc.tensor.transpose(x_ps[:], st16[:], ident[0:16, 0:16])
                return x_ps

            x3_ps = all_reduce(ar1_in, "ar1")
            x3 = small.tile([128, 16], dt, tag="x3")
            nc.vector.tensor_add(x3[:], x3_ps[:], x1[:])  # + residual

            # ---- MLP1: h = relu(w1 @ x4) via 2 PE chains ----
            x4 = rms(x3, "n3")
            x4b = to_bf(x4, "x4")
            ph = [ps.tile([1, 512], dt, tag="b", name=f"ph{n}") for n in range(2)]
            for a in range(4):
                w1t = stile("w1")
                nc.sync.dma_start(w1t[:], w1_r[:, a * 4096 : (a + 1) * 4096])
                for b in range(4):
                    k = a * 4 + b
                    for n in range(2):
                        nc.tensor.matmul(
                            ph[n][:], x4b[:, k : k + 1],
                            w1t[:, b * 1024 + n * 512 : b * 1024 + (n + 1) * 512],
                            start=(k == 0), stop=(k == 15),
                        )
            h_row = small.tile([1, 1024], dt, tag="hrow")
            for n in range(2):
                nc.scalar.activation(h_row[:, n * 512 : (n + 1) * 512], ph[n][:], AF.Relu)

            st8 = stage.tile([8, 128], dt, tag="st8")
            nc.scalar.dma_start(st8[:], h_row[:])
            hT_ps = ps.tile([128, 8], dt, tag="b")
            nc.tensor.transpose(hT_ps[:], st8[:], ident[0:8, 0:8])
            hTb = small.tile([128, 8], bf, tag="hTb")
            nc.vector.tensor_copy(hTb[:], hT_ps[:])

            # ---- MLP2: [1, 2048] partial via 4 PE chains ----
            ar2_in = small.tile([1, 2048], dt, tag="arin", name="ar2_in")
            pm = [ps.tile([1, 512], dt, tag="b", name=f"pm{n}") for n in range(4)]
            for a in range(4):
                w2t = stile("w2")
                nc.sync.dma_start(w2t[:], w2_r[:, a * 4096 : (a + 1) * 4096])
                for b in range(2):
                    k = a * 2 + b
                    for n in range(4):
                        nc.tensor.matmul(
                            pm[n][:], hTb[:, k : k + 1],
                            w2t[:, b * 2048 + n * 512 : b * 2048 + (n + 1) * 512],
                            start=(k == 0), stop=(k == 7),
                        )
            for n in range(4):
                nc.vector.tensor_copy(ar2_in[:, n * 512 : (n + 1) * 512], pm[n][:])

            x5_ps = all_reduce(ar2_in, "ar2")
            x5 = small.tile([128, 16], dt, tag="x5")
            nc.vector.tensor_add(x5[:], x5_ps[:], x3[:])  # + residual (x3)
            x5b = to_bf(x5, "x5")

            # ---- LM head over the vocab shard, two passes. Per pass/k one DMA
            # of [128, width]; DVE accumulates cols 0:1024 via stt, PE chains
            # the rest in N<=512 chunks.
            passes = [(0, 3328), (3328, VPC)]
            DVE_W = 768
            for lo, hi in passes:
                width = hi - lo
                acc_lm = accp.tile([128, DVE_W], dt, tag="acclm", name=f"acc_lm{lo}")
                pe_w = width - DVE_W
                npe = (pe_w + 511) // 512
                pe_ps = [
                    ps.tile(
                        [1, min(512, pe_w - 512 * i)], dt, tag="b", name=f"pe{lo}_{i}"
                    )
                    for i in range(npe)
                ]
                for k in range(16):
                    lt = stile("lm", 3328)
                    nc.sync.dma_start(
                        lt[:, 0:width], lm_r[:, k * VPC + lo : k * VPC + hi]
                    )
                    if k == 0:
                        nc.vector.tensor_scalar_mul(
                            acc_lm[:], lt[:, 0:DVE_W], x5[:, 0:1]
                        )
                    else:
                        nc.vector.scalar_tensor_tensor(
                            acc_lm[:], lt[:, 0:DVE_W], x5[:, k : k + 1], acc_lm[:],
                            op0=MUL, op1=ADD,
                        )
                    for i in range(npe):
                        cw = pe_ps[i].shape[1]
                        nc.tensor.matmul(
                            pe_ps[i][:], x5b[:, k : k + 1],
                            lt[:, DVE_W + i * 512 : DVE_W + i * 512 + cw],
                            start=(k == 0), stop=(k == 15),
                        )
                ndr = (DVE_W + 511) // 512
                for n in range(ndr):
                    dw = min(512, DVE_W - n * 512)
                    pl = ps.tile([1, 512], dt, tag="b", name=f"pla{lo}_{n}")
                    nc.tensor.matmul(
                        pl[:, 0:dw], ones_col[:], acc_lm[:, n * 512 : n * 512 + dw],
                        start=True, stop=True,
                    )
                    lr = small.tile([1, 512], dt, tag="lrow", name=f"lr{lo}_{n}", bufs=3)
                    nc.vector.tensor_copy(lr[:, 0:dw], pl[:, 0:dw])
                    if lo == 0 and n == 0:
                        # keep the warmup-collective result alive off the
                        # critical path: lr[0, :16] += 0 * warm
                        nc.vector.scalar_tensor_tensor(
                            lr[0:1, 0:16], warm_back[:], 0.0, lr[0:1, 0:16],
                            op0=MUL, op1=ADD,
                        )
                    nc.scalar.dma_start(
                        logits_out[:, lo + n * 512 : lo + n * 512 + dw], lr[:, 0:dw]
                    )
                for i in range(npe):
                    cw = pe_ps[i].shape[1]
                    lr = small.tile(
                        [1, 512], dt, tag="lrow", name=f"lrp{lo}_{i}", bufs=3
                    )
                    nc.vector.tensor_copy(lr[:, 0:cw], pe_ps[i][:])
                    nc.scalar.dma_start(
                        logits_out[:, lo + DVE_W + i * 512 : lo + DVE_W + i * 512 + cw],
                        lr[:, 0:cw],
                    )

    nc.finalize()
    return nc


def _col16(v):
    """[2048] vector -> [128, 16] column-major layout (e = c*128 + p at [p, c])."""
    return np.ascontiguousarray(v.reshape(16, 128).T)


def _part_major(mT, nblk, blk_rows, width):
    """[nblk*blk_rows, width] -> [blk_rows, nblk*width] partition-major, bf16."""
    out = mT.reshape(nblk, blk_rows, width).transpose(1, 0, 2).reshape(
        blk_rows, nblk * width
    )
    return np.ascontiguousarray(out.astype(ml_dtypes.bfloat16))


def _prep_in_maps(token_id, pos_id, keys, values, wte, wpe, wq, wk, wv, wo, w1, w2, lm_w):
    f32 = lambda a: np.asarray(a, dtype=np.float32)
    keys, values = f32(keys), f32(values)
    wq, wk, wv, wo, w1, w2, lm_w = map(f32, (wq, wk, wv, wo, w1, w2, lm_w))
    wq = wq * np.float32(SCALE)  # fold attention scale into q
    xe_wte = _col16(f32(wte[token_id]))
    xe_wpe = _col16(f32(wpe[pos_id]))
    lm_pad = np.zeros((N_CORES * VPC, E), np.float32)
    lm_pad[:VOCAB] = lm_w

    in_maps = []
    for i in range(N_CORES):
        hs = slice(i * EPC, (i + 1) * EPC)
        wqkv = np.concatenate([wq[hs], wk[hs], wv[hs]], axis=0)  # [768, E]
        in_maps.append(
            {
                "xe_wte": xe_wte,
                "xe_wpe": xe_wpe,
                "wqkv_r": _part_major(np.ascontiguousarray(wqkv.T), 16, 128, 768),
                "keys_r": _part_major(np.ascontiguousarray(keys[:, hs].T), 2, 128, 8192),
                "vals_r": _part_major(values[:, hs], 64, 128, EPC),
                "wo_r": _part_major(np.ascontiguousarray(wo[:, hs].T), 2, 128, E),
                "w1_r": _part_major(
                    np.ascontiguousarray(w1[i * 1024 : (i + 1) * 1024].T), 16, 128, 1024
                ),
                "w2_r": _part_major(
                    np.ascontiguousarray(w2[:, i * 1024 : (i + 1) * 1024].T), 8, 128, E
                ),
                "lm_r": _part_major(
                    np.ascontiguousarray(lm_pad[i * VPC : (i + 1) * VPC].T), 16, 128, VPC
                ),
            }
        )
    return in_maps


def kernel(**inputs) -> np.ndarray:
    from concourse.bass_utils import run_bass_kernel_spmd

    token_id = int(inputs["token_id"])
    pos_id = int(inputs["pos_id"])
    in_maps = _prep_in_maps(
        token_id,
        pos_id,
        inputs["keys"],
        inputs["values"],
        inputs["wte"],
        inputs["wpe"],
        inputs["wq"],
        inputs["wk"],
        inputs["wv"],
        inputs["wo"],
        inputs["w1"],
        inputs["w2"],
        inputs["lm_w"],
    )
    if "nc" not in _CACHE:
        _CACHE["nc"] = _build_nc()
    nc = _CACHE["nc"]
    res = run_bass_kernel_spmd(
        nc,
        in_maps,
        core_ids=list(range(N_CORES)),
        trace=TRACE,
        trace_cores=[0] if TRACE else None,
    )
    _CACHE["last_result"] = res
    logits = np.concatenate([r["logits"][0] for r in res.results])[:VOCAB]
    return np.ascontiguousarray(logits.astype(np.float32))


# revision 12
# speedup vs baseline: 1.9026x; 1.0390x over previous
"""Tensor-parallel MiniGPT single-token decode step on 8 Trainium2 NeuronCores.

Sharding (per core i of 8):
  - attention: heads 2i, 2i+1 (head_dim 128 -> cols i*256:(i+1)*256 of E=2048);
    wq/wk/wv row-sharded, wo column-sharded, KV cache column-sharded by head.
  - MLP: w1 row-sharded (1024 rows/core), w2 column-sharded.
  - LM head: vocab-sharded (50257 padded to 8*6283=50264 rows).
  - Two 8KB AllReduces combine the wo- and w2- partial sums; logits are
    gathered on the host.

All weights are cast to bf16 on the host and laid out as [128, F]
partition-major arrays, so every device DMA is one contiguous ~1-1.6MB run at
half the fp32 byte count. Activations stay fp32; PE matmuls take bf16 operands
and accumulate fp32 in PSUM.

Matvec strategy: PE does nearly everything. x-chunks are the [128, 1]
stationary operand, weight tiles stream as the moving operand in N<=512 chunks
with PSUM accumulation across k-blocks. Output chunks are spread across PSUM
*partition rows* 0/32/64/96 (PE column tiling via out.base_partition), so up
to 4 chains run concurrently in different column groups of the array and 4
chains share one PSUM bank. Attention scores are computed directly in column
layout (key-block stationary, q moving); exp runs on ACT straight from PSUM
with accum_out providing softmax denominators. Row->column transposes use K=1
matmuls (lhsT = the row, rhs = a [1,1] one) instead of DMA reshapes. The
collective path is warmed with an early AllReduce whose result is consumed at
the output tail.
"""

import numpy as np
import ml_dtypes

N_CORES = 8
E = 2048
HPC = 2  # heads per core
EPC = HPC * 128  # 256
T = 8192
VOCAB = 50257
VPC = 6283  # padded vocab rows per core (8 * 6283 = 50264)
SCALE = float(1.0 / np.sqrt(128.0))
EPS = 1e-5

_CACHE = {}
TRACE = False


def _build_nc():
    import concourse.bacc as bacc
    import concourse.mybir as mybir
    import concourse.tile as tile

    AF = mybir.ActivationFunctionType
    MUL = mybir.AluOpType.mult
    ADD = mybir.AluOpType.add
    dt = mybir.dt.float32
    bf = mybir.dt.bfloat16

    nc = bacc.Bacc(
        "TRN2", target_bir_lowering=False, debug=False, num_devices=N_CORES
    )

    xe_wte = nc.declare_dram_parameter("xe_wte", [128, 16], dt, isOutput=False)
    xe_wpe = nc.declare_dram_parameter("xe_wpe", [128, 16], dt, isOutput=False)
    wqkv_r = nc.declare_dram_parameter("wqkv_r", [128, 16 * 768], bf, isOutput=False)
    keys_r = nc.declare_dram_parameter("keys_r", [128, 2 * 8192], bf, isOutput=False)
    vals_r = nc.declare_dram_parameter("vals_r", [128, 64 * 256], bf, isOutput=False)
    wo_r = nc.declare_dram_parameter("wo_r", [128, 2 * 2048], bf, isOutput=False)
    w1_r = nc.declare_dram_parameter("w1_r", [128, 16 * 1024], bf, isOutput=False)
    w2_r = nc.declare_dram_parameter("w2_r", [128, 8 * 2048], bf, isOutput=False)
    lm_r = nc.declare_dram_parameter("lm_r", [128, 16 * VPC], bf, isOutput=False)
    logits_out = nc.declare_dram_parameter("logits", [1, VPC], dt, isOutput=True)

    with tile.TileContext(nc) as tc:
        with (
            tc.tile_pool(name="const", bufs=1) as const,
            tc.tile_pool(name="small", bufs=1) as small,
            tc.tile_pool(name="stage", bufs=2) as stage,
            tc.tile_pool(name="ps", bufs=4, space="PSUM") as ps,
            tc.tile_pool(name="dram", bufs=1, space="DRAM") as dram,
            tc.tile_pool(name="stream", bufs=13) as stream,
        ):
            _snum = [0]

            def stile(label, width):
                _snum[0] += 1
                return stream.tile(
                    [128, width], bf, tag="s", name=f"s{_snum[0]}_{label}"
                )

            # Warm up the collectives path first: CC mesh init (~67us) starts
            # at the first collective trigger, and the first AllReduce runs
            # ~2x slower than later ones, so pay both early. The result is
            # consumed (x0) at the output tail only.
            warm_row = small.tile([1, 2048], dt, tag="arin", name="warm_row")
            nc.vector.memset(warm_row[:], 0.0)
            warm_in = dram.tile([1, 2048], dt, tag="warm_in")
            warm_out = dram.tile([1, 2048], dt, tag="warm_out")
            nc.scalar.dma_start(warm_in[:], warm_row[:])
            nc.gpsimd.collective_compute(
                "AllReduce",
                ADD,
                replica_groups=[list(range(N_CORES))],
                ins=[warm_in.opt()],
                outs=[warm_out.opt()],
            )
            warm_back = stage.tile([1, 16], dt, tag="warmb", bufs=1)
            nc.gpsimd.dma_start(warm_back[:], warm_out[0:1, 0:16])

            ones_col = const.tile([128, 1], dt)
            nc.vector.memset(ones_col[:], 1.0)
            ones_row = const.tile([1, 128], dt)
            nc.vector.memset(ones_row[:], 1.0)
            one1 = const.tile([1, 1], dt)
            nc.vector.memset(one1[:], 1.0)
            eps_c = const.tile([1, 1], dt)
            nc.vector.memset(eps_c[:], EPS)
            junk = small.tile([1, 1], dt, tag="junk")
            # preload the ACT Sqrt LUT off the critical path
            nc.scalar.sqrt(junk[:], eps_c[:])

            def rms(xt, name):
                """x * rsqrt(mean(x^2) + eps) for x in [128, 16] column layout."""
                sq = small.tile([128, 16], dt, tag=f"sq_{name}")
                ssum = small.tile([128, 1], dt, tag=f"ss_{name}")
                nc.scalar.activation(sq[:], xt[:], AF.Square, accum_out=ssum[:])
                tot = ps.tile([1, 1], dt, tag="b")
                nc.tensor.matmul(tot[:], ssum[:], ones_col[:], start=True, stop=True)
                std = small.tile([1, 1], dt, tag=f"std_{name}")
                nc.scalar.activation(
                    std[:], tot[:], AF.Sqrt, bias=eps_c[:], scale=1.0 / float(E)
                )
                inv = small.tile([1, 1], dt, tag=f"inv_{name}")
                nc.vector.reciprocal(inv[:], std[:])
                invb_ps = ps.tile([128, 1], dt, tag="b")
                nc.tensor.matmul(
                    invb_ps[:], ones_row[:], inv[:], start=True, stop=True
                )
                xn = small.tile([128, 16], dt, tag=f"xn_{name}")
                nc.vector.tensor_scalar_mul(xn[:], xt[:], invb_ps[:])
                return xn

            def to_bf(xt, name, w=16):
                xb = small.tile([128, w], bf, tag=f"bf_{name}")
                nc.vector.tensor_copy(xb[:], xt[:])
                return xb

            def row_to_col(row_sb, nblk, name):
                """[1, nblk*128] row (on one partition) -> PSUM [128, nblk]
                columns via nblk K=1 matmuls (no DMA round-trip)."""
                out = ps.tile([128, nblk], dt, tag="b", name=f"r2c_{name}")
                for c in range(nblk):
                    nc.tensor.matmul(
                        out[:, c : c + 1],
                        row_sb[0:1, c * 128 : (c + 1) * 128],
                        one1[:],
                        start=True, stop=True,
                    )
                return out

            # ---- embedding + double rms ----
            xw = stage.tile([128, 16], dt, tag="xw")
            nc.scalar.dma_start(xw[:], xe_wte[:])
            xp = stage.tile([128, 16], dt, tag="xp")
            nc.scalar.dma_start(xp[:], xe_wpe[:])
            x0 = small.tile([128, 16], dt, tag="x0")
            nc.vector.tensor_add(x0[:], xw[:], xp[:])
            x1 = rms(x0, "n1")  # residual input
            x2 = rms(x1, "n2")
            x2b = to_bf(x2, "x2")

            # ---- qkv projection: [1, 768] row (q 0:256 | k 256:512 | v 512:768)
            # SCALE is folded into wq on the host. Two col-tiled chains
            # (rows 0 and 32 of one PSUM bank) run concurrently on PE.
            qkv_ps = ps.tile([128, 512], dt, tag="b", name="qkv_ps")
            for t in range(4):
                wt = stile("qkv", 3072)
                nc.sync.dma_start(wt[:], wqkv_r[:, t * 3072 : (t + 1) * 3072])
                for b in range(4):
                    k = 4 * t + b
                    nc.tensor.matmul(
                        qkv_ps[0:1, 0:512], x2b[:, k : k + 1],
                        wt[:, b * 768 : b * 768 + 512],
                        start=(k == 0), stop=(k == 15),
                    )
                    nc.tensor.matmul(
                        qkv_ps[32:33, 0:256], x2b[:, k : k + 1],
                        wt[:, b * 768 + 512 : b * 768 + 768],
                        start=(k == 0), stop=(k == 15),
                        tile_position=(0, 32),
                    )
            qkv_row = small.tile([1, 768], dt, tag="qkv")
            nc.scalar.copy(qkv_row[:, 0:512], qkv_ps[0:1, 0:512])
            nc.scalar.copy(qkv_row[:, 512:768], qkv_ps[32:33, 0:256])

            # ---- q,k to column layout via K=1 matmuls ----
            qkT_ps = row_to_col(qkv_row, 4, "qk")
            qkTb = small.tile([128, 4], bf, tag="qkTb")
            nc.vector.tensor_copy(qkTb[:], qkT_ps[:])

            # ---- attention scores in column layout: per head h and 128-row
            # t-block c, matmul(out[128,1], lhsT=K_block[128d,128t], rhs=q[128,1])
            # gives scores for t in [c*128, (c+1)*128) on partitions. ACT exp
            # straight from PSUM, accum_out accumulates softmax denominators.
            wTs = []
            esp = small.tile([128, 4], dt, tag="esp")  # per-partition exp sums
            for h in range(HPC):
                wTs.append(small.tile([128, 64], bf, tag=f"wT{h}", name=f"wT{h}"))
            for h in range(HPC):
                for j in range(2):
                    kt = stile("key", 4096)
                    nc.sync.dma_start(
                        kt[:],
                        keys_r[:, h * 8192 + j * 4096 : h * 8192 + (j + 1) * 4096],
                    )
                    qk_ps = ps.tile([128, 32], dt, tag="b", name=f"qk{h}_{j}")
                    for c in range(32):
                        nc.tensor.matmul(
                            qk_ps[:, c : c + 1],
                            kt[:, c * 128 : (c + 1) * 128],
                            qkTb[:, h : h + 1],
                            start=True, stop=True,
                        )
                    nc.scalar.activation(
                        wTs[h][:, j * 32 : (j + 1) * 32], qk_ps[:], AF.Exp,
                        accum_out=esp[:, h * 2 + j : h * 2 + j + 1],
                    )

            # current-token score per head: exp(q_h . k_h) (SCALE folded into q)
            e_last = small.tile([1, 2], dt, tag="elast")
            for h in range(HPC):
                pal = ps.tile([1, 1], dt, tag="b")
                nc.tensor.matmul(
                    pal[:], qkTb[:, h : h + 1], qkTb[:, 2 + h : 3 + h],
                    start=True, stop=True,
                )
                nc.scalar.activation(e_last[:, h : h + 1], pal[:], AF.Exp)

            # softmax denominators: cross-partition sum of esp + e_last
            dps = ps.tile([1, 4], dt, tag="b")
            nc.tensor.matmul(dps[:], ones_col[:], esp[:], start=True, stop=True)
            dtmp = small.tile([1, 2], dt, tag="dtmp")
            for h in range(HPC):
                nc.vector.reduce_sum(
                    dtmp[:, h : h + 1], dps[:, h * 2 : (h + 1) * 2],
                    axis=mybir.AxisListType.X,
                )
            nc.vector.tensor_add(dtmp[:], dtmp[:], e_last[:])
            dinv = small.tile([1, 2], dt, tag="dinv")
            nc.vector.reciprocal(dinv[:], dtmp[:])

            # ---- PV on PE: x_attn_h = sum_t w[t] * V[t, :] (unnormalized).
            # Per t-block c: matmul(out[1,128], lhsT=wT[:, c], rhs=V_block),
            # accumulating over all 64 blocks; the two heads run in different
            # PE column groups (rows 0 / 32 of one bank).
            pv_ps = ps.tile([128, 128], dt, tag="b", name="pv_ps")
            for tt in range(4):
                vt = stile("val", 4096)
                nc.sync.dma_start(vt[:], vals_r[:, tt * 4096 : (tt + 1) * 4096])
                for b in range(16):
                    c = tt * 16 + b
                    for h in range(HPC):
                        nc.tensor.matmul(
                            pv_ps[32 * h : 32 * h + 1, :],
                            wTs[h][:, c : c + 1],
                            vt[:, b * 256 + h * 128 : b * 256 + (h + 1) * 128],
                            start=(c == 0), stop=(c == 63),
                            tile_position=(0, 32 * h),
                        )

            # combine with current-token value, then normalize by the softmax sum
            xa_row = small.tile([1, 256], dt, tag="xa")
            for h in range(HPC):
                sl = slice(h * 128, (h + 1) * 128)
                nc.vector.tensor_scalar_mul(
                    xa_row[:, sl],
                    qkv_row[:, 512 + h * 128 : 512 + (h + 1) * 128],
                    e_last[:, h : h + 1],
                )
                nc.vector.tensor_add(
                    xa_row[:, sl], xa_row[:, sl], pv_ps[32 * h : 32 * h + 1, :]
                )
                nc.vector.tensor_scalar_mul(xa_row[:, sl], xa_row[:, sl], dinv[:, h : h + 1])

            xaT_ps = row_to_col(xa_row, 2, "xa")
            xaTb = small.tile([128, 2], bf, tag="xaTb")
            nc.vector.tensor_copy(xaTb[:], xaT_ps[:])

            # ---- wo partial: [1, 2048] via 4 col-tiled PE chains ----
            ar1_in = small.tile([1, 2048], dt, tag="arin", name="ar1_in")
            wo_ps = ps.tile([128, 512], dt, tag="b", name="wo_ps")
            wot = stile("wo", 4096)
            nc.sync.dma_start(wot[:], wo_r[:])
            for k in range(2):
                for n in range(4):
                    nc.tensor.matmul(
                        wo_ps[32 * n : 32 * n + 1, :], xaTb[:, k : k + 1],
                        wot[:, k * 2048 + n * 512 : k * 2048 + (n + 1) * 512],
                        start=(k == 0), stop=(k == 1),
                        tile_position=(0, 32 * n),
                    )
            for n in range(4):
                eng = nc.vector.tensor_copy if n % 2 == 0 else nc.scalar.copy
                eng(ar1_in[:, n * 512 : (n + 1) * 512], wo_ps[32 * n : 32 * n + 1, :])

            def all_reduce(row_sb, name):
                """Sum a [1, 2048] partial across cores; returns PSUM [128, 16].

                CCE AllReduce in DRAM, readback as [4, 512], then 16 K=1
                matmuls rebuild the [128, 16] column layout (no PE transpose,
                no reshape DMA).
                """
                in_d = dram.tile([1, 2048], dt, tag=f"{name}_in")
                out_d = dram.tile([1, 2048], dt, tag=f"{name}_out")
                nc.scalar.dma_start(in_d[:], row_sb[:])
                nc.gpsimd.collective_compute(
                    "AllReduce",
                    ADD,
                    replica_groups=[list(range(N_CORES))],
                    ins=[in_d.opt()],
                    outs=[out_d.opt()],
                )
                rb = stage.tile([1, 2048], dt, tag="rb", name=f"rb_{name}", bufs=1)
                nc.scalar.dma_start(rb[:], out_d[:])
                return row_to_col(rb, 16, name)

            x3_ps = all_reduce(ar1_in, "ar1")
            # re-preload the Sqrt LUT while the AllReduce is in flight
            nc.scalar.sqrt(junk[:], eps_c[:])
            x3 = small.tile([128, 16], dt, tag="x3")
            nc.vector.tensor_add(x3[:], x3_ps[:], x1[:])  # + residual

            # ---- MLP1: h = relu(w1 @ x4) via 2 col-tiled PE chains ----
            x4 = rms(x3, "n3")
            x4b = to_bf(x4, "x4")
            mh_ps = ps.tile([128, 512], dt, tag="b", name="mh_ps")
            for a in range(4):
                w1t = stile("w1", 4096)
                nc.sync.dma_start(w1t[:], w1_r[:, a * 4096 : (a + 1) * 4096])
                for b in range(4):
                    k = a * 4 + b
                    for n in range(2):
                        nc.tensor.matmul(
                            mh_ps[32 * n : 32 * n + 1, :], x4b[:, k : k + 1],
                            w1t[:, b * 1024 + n * 512 : b * 1024 + (n + 1) * 512],
                            start=(k == 0), stop=(k == 15),
                            tile_position=(0, 32 * n),
                        )
            h_row = small.tile([1, 1024], dt, tag="hrow")
            for n in range(2):
                nc.scalar.activation(
                    h_row[:, n * 512 : (n + 1) * 512],
                    mh_ps[32 * n : 32 * n + 1, :], AF.Relu,
                )

            hT_ps = row_to_col(h_row, 8, "h")
            hTb = small.tile([128, 8], bf, tag="hTb")
            nc.vector.tensor_copy(hTb[:], hT_ps[:])

            # ---- MLP2: [1, 2048] partial via 4 col-tiled PE chains ----
            ar2_in = small.tile([1, 2048], dt, tag="arin", name="ar2_in")
            m2_ps = ps.tile([128, 512], dt, tag="b", name="m2_ps")
            for a in range(4):
                w2t = stile("w2", 4096)
                nc.sync.dma_start(w2t[:], w2_r[:, a * 4096 : (a + 1) * 4096])
                for b in range(2):
                    k = a * 2 + b
                    for n in range(4):
                        nc.tensor.matmul(
                            m2_ps[32 * n : 32 * n + 1, :], hTb[:, k : k + 1],
                            w2t[:, b * 2048 + n * 512 : b * 2048 + (n + 1) * 512],
                            start=(k == 0), stop=(k == 7),
                            tile_position=(0, 32 * n),
                        )
            for n in range(4):
                eng = nc.vector.tensor_copy if n % 2 == 0 else nc.scalar.copy
                eng(ar2_in[:, n * 512 : (n + 1) * 512], m2_ps[32 * n : 32 * n + 1, :])

            x5_ps = all_reduce(ar2_in, "ar2")
            x5 = small.tile([128, 16], dt, tag="x5")
            nc.vector.tensor_add(x5[:], x5_ps[:], x3[:])  # + residual (x3)
            x5b = to_bf(x5, "x5")

            # ---- LM head over the vocab shard: a single k-loop of full-width
            # [128, 6283] tiles. 13 output chunks as col-tiled PE chains, 4 per
            # PSUM bank (partition rows 0/32/64/96), all accumulating over the
            # 16 k-blocks concurrently.
            NCH = (VPC + 511) // 512  # 13
            banks = [
                ps.tile([128, 512], dt, tag="lmb", name=f"lmb{b}", bufs=4)
                for b in range((NCH + 3) // 4)
            ]

            def chain_out(c, cw):
                return banks[c // 4][32 * (c % 4) : 32 * (c % 4) + 1, 0:cw]

            for k in range(16):
                lt = stile("lm", VPC)
                nc.sync.dma_start(lt[:], lm_r[:, k * VPC : (k + 1) * VPC])
                for c in range(NCH):
                    cw = min(512, VPC - c * 512)
                    nc.tensor.matmul(
                        chain_out(c, cw), x5b[:, k : k + 1],
                        lt[:, c * 512 : c * 512 + cw],
                        start=(k == 0), stop=(k == 15),
                        tile_position=(0, 32 * (c % 4)),
                    )
            for c in range(NCH):
                cw = min(512, VPC - c * 512)
                lr = small.tile([1, 512], dt, tag="lrow", name=f"lr{c}", bufs=4)
                if c % 2 == 0:
                    nc.vector.tensor_copy(lr[:, 0:cw], chain_out(c, cw))
                else:
                    nc.scalar.copy(lr[:, 0:cw], chain_out(c, cw))
                if c == 0:
                    # keep the warmup-collective result alive off the
                    # critical path: lr[0, :16] += 0 * warm
                    nc.vector.scalar_tensor_tensor(
                        lr[0:1, 0:16], warm_back[:], 0.0, lr[0:1, 0:16],
                        op0=MUL, op1=ADD,
                    )
                nc.scalar.dma_start(
                    logits_out[:, c * 512 : c * 512 + cw], lr[:, 0:cw]
                )

    nc.finalize()
    return nc


def _col16(v):
    """[2048] vector -> [128, 16] column-major layout (e = c*128 + p at [p, c])."""
    return np.ascontiguousarray(v.reshape(16, 128).T)


def _part_major(mT, nblk, blk_rows, width):
    """[nblk*blk_rows, width] -> [blk_rows, nblk*width] partition-major, bf16."""
    out = mT.reshape(nblk, blk_rows, width).transpose(1, 0, 2).reshape(
        blk_rows, nblk * width
    )
    return np.ascontiguousarray(out.astype(ml_dtypes.bfloat16))


def _prep_in_maps(token_id, pos_id, keys, values, wte, wpe, wq, wk, wv, wo, w1, w2, lm_w):
    f32 = lambda a: np.asarray(a, dtype=np.float32)
    keys, values = f32(keys), f32(values)
    wq, wk, wv, wo, w1, w2, lm_w = map(f32, (wq, wk, wv, wo, w1, w2, lm_w))
    wq = wq * np.float32(SCALE)  # fold attention scale into q
    xe_wte = _col16(f32(wte[token_id]))
    xe_wpe = _col16(f32(wpe[pos_id]))
    lm_pad = np.zeros((N_CORES * VPC, E), np.float32)
    lm_pad[:VOCAB] = lm_w

    in_maps = []
    for i in range(N_CORES):
        hs = slice(i * EPC, (i + 1) * EPC)
        wqkv = np.concatenate([wq[hs], wk[hs], wv[hs]], axis=0)  # [768, E]
        in_maps.append(
            {
                "xe_wte": xe_wte,
                "xe_wpe": xe_wpe,
                "wqkv_r": _part_major(np.ascontiguousarray(wqkv.T), 16, 128, 768),
                "keys_r": _part_major(np.ascontiguousarray(keys[:, hs].T), 2, 128, 8192),
                "vals_r": _part_major(values[:, hs], 64, 128, EPC),
                "wo_r": _part_major(np.ascontiguousarray(wo[:, hs].T), 2, 128, E),
                "w1_r": _part_major(
                    np.ascontiguousarray(w1[i * 1024 : (i + 1) * 1024].T), 16, 128, 1024
                ),
                "w2_r": _part_major(
                    np.ascontiguousarray(w2[:, i * 1024 : (i + 1) * 1024].T), 8, 128, E
                ),
                "lm_r": _part_major(
                    np.ascontiguousarray(lm_pad[i * VPC : (i + 1) * VPC].T), 16, 128, VPC
                ),
            }
        )
    return in_maps


def kernel(**inputs) -> np.ndarray:
    from concourse.bass_utils import run_bass_kernel_spmd

    token_id = int(inputs["token_id"])
    pos_id = int(inputs["pos_id"])
    in_maps = _prep_in_maps(
        token_id,
        pos_id,
        inputs["keys"],
        inputs["values"],
        inputs["wte"],
        inputs["wpe"],
        inputs["wq"],
        inputs["wk"],
        inputs["wv"],
        inputs["wo"],
        inputs["w1"],
        inputs["w2"],
        inputs["lm_w"],
    )
    if "nc" not in _CACHE:
        _CACHE["nc"] = _build_nc()
    nc = _CACHE["nc"]
    res = run_bass_kernel_spmd(
        nc,
        in_maps,
        core_ids=list(range(N_CORES)),
        trace=TRACE,
        trace_cores=[0] if TRACE else None,
    )
    _CACHE["last_result"] = res
    logits = np.concatenate([r["logits"][0] for r in res.results])[:VOCAB]
    return np.ascontiguousarray(logits.astype(np.float32))


# revision 13
# speedup vs baseline: 1.9610x; 1.0307x over previous
"""Tensor-parallel MiniGPT single-token decode step on 8 Trainium2 NeuronCores.

Sharding (per core i of 8):
  - attention: heads 2i, 2i+1 (head_dim 128 -> cols i*256:(i+1)*256 of E=2048);
    wq/wk/wv row-sharded, wo column-sharded, KV cache column-sharded by head.
  - MLP: w1 row-sharded (1024 rows/core), w2 column-sharded.
  - LM head: vocab-sharded (50257 padded to 8*6283=50264 rows).
  - Two 8KB AllReduces combine the wo- and w2- partial sums; logits are
    gathered on the host.

All weights are cast to bf16 on the host and laid out as [128, F]
partition-major arrays, so every device DMA is one contiguous ~1-1.6MB run at
half the fp32 byte count. Activations stay fp32; PE matmuls take bf16 operands
and accumulate fp32 in PSUM.

Matvec strategy: PE does nearly everything. x-chunks are the [128, 1]
stationary operand, weight tiles stream as the moving operand in N<=512 chunks
with PSUM accumulation across k-blocks. Output chunks are spread across PSUM
*partition rows* 0/32/64/96 (PE column tiling via out.base_partition), so up
to 4 chains run concurrently in different column groups of the array and 4
chains share one PSUM bank. Attention scores are computed directly in column
layout (key-block stationary, q moving); exp runs on ACT straight from PSUM
with accum_out providing softmax denominators. Row->column transposes use K=1
matmuls (lhsT = the row, rhs = a [1,1] one) instead of DMA reshapes. The
collective path is warmed with an early AllReduce whose result is consumed at
the output tail.
"""

import numpy as np
import ml_dtypes

N_CORES = 8
E = 2048
HPC = 2  # heads per core
EPC = HPC * 128  # 256
T = 8192
VOCAB = 50257
VPC = 6283  # padded vocab rows per core (8 * 6283 = 50264)
SCALE = float(1.0 / np.sqrt(128.0))
EPS = 1e-5

_CACHE = {}
TRACE = False


def _build_nc():
    import concourse.bacc as bacc
    import concourse.mybir as mybir
    import concourse.tile as tile

    AF = mybir.ActivationFunctionType
    MUL = mybir.AluOpType.mult
    ADD = mybir.AluOpType.add
    dt = mybir.dt.float32
    bf = mybir.dt.bfloat16

    nc = bacc.Bacc(
        "TRN2", target_bir_lowering=False, debug=False, num_devices=N_CORES
    )

    xe_wte = nc.declare_dram_parameter("xe_wte", [128, 16], dt, isOutput=False)
    xe_wpe = nc.declare_dram_parameter("xe_wpe", [128, 16], dt, isOutput=False)
    wqkv_r = nc.declare_dram_parameter("wqkv_r", [128, 16 * 768], bf, isOutput=False)
    keys_r = nc.declare_dram_parameter("keys_r", [128, 2 * 8192], bf, isOutput=False)
    vals_r = nc.declare_dram_parameter("vals_r", [128, 64 * 256], bf, isOutput=False)
    wo_r = nc.declare_dram_parameter("wo_r", [128, 2 * 2048], bf, isOutput=False)
    w1_r = nc.declare_dram_parameter("w1_r", [128, 16 * 1024], bf, isOutput=False)
    w2_r = nc.declare_dram_parameter("w2_r", [128, 8 * 2048], bf, isOutput=False)
    lm_r = nc.declare_dram_parameter("lm_r", [128, 16 * VPC], bf, isOutput=False)
    logits_out = nc.declare_dram_parameter("logits", [1, VPC], dt, isOutput=True)

    with tile.TileContext(nc) as tc:
        with (
            tc.tile_pool(name="const", bufs=1) as const,
            tc.tile_pool(name="small", bufs=1) as small,
            tc.tile_pool(name="stage", bufs=2) as stage,
            tc.tile_pool(name="ps", bufs=4, space="PSUM") as ps,
            tc.tile_pool(name="dram", bufs=1, space="DRAM") as dram,
            tc.tile_pool(name="stream", bufs=13) as stream,
        ):
            _snum = [0]

            def stile(label, width):
                _snum[0] += 1
                return stream.tile(
                    [128, width], bf, tag="s", name=f"s{_snum[0]}_{label}"
                )

            # Warm up the collectives path first: CC mesh init (~67us) starts
            # at the first collective trigger, and the first AllReduce runs
            # ~2x slower than later ones, so pay both early. The result is
            # consumed (x0) at the output tail only.
            warm_row = small.tile([1, 2048], dt, tag="arin", name="warm_row")
            nc.vector.memset(warm_row[:], 0.0)
            warm_in = dram.tile([1, 2048], dt, tag="warm_in")
            warm_out = dram.tile([1, 2048], dt, tag="warm_out")
            nc.scalar.dma_start(warm_in[:], warm_row[:])
            nc.gpsimd.collective_compute(
                "AllReduce",
                ADD,
                replica_groups=[list(range(N_CORES))],
                ins=[warm_in.opt()],
                outs=[warm_out.opt()],
            )
            warm_back = stage.tile([1, 16], dt, tag="warmb", bufs=1)
            nc.gpsimd.dma_start(warm_back[:], warm_out[0:1, 0:16])

            ones_col = const.tile([128, 1], dt)
            nc.vector.memset(ones_col[:], 1.0)
            ones_row = const.tile([1, 128], dt)
            nc.vector.memset(ones_row[:], 1.0)
            one1 = const.tile([1, 1], dt)
            nc.vector.memset(one1[:], 1.0)
            eps_c = const.tile([1, 1], dt)
            nc.vector.memset(eps_c[:], EPS)
            junk = small.tile([1, 1], dt, tag="junk")
            # preload the ACT Sqrt LUT off the critical path
            nc.scalar.sqrt(junk[:], eps_c[:])

            def rms(xt, name, out_bf=False):
                """x * rsqrt(mean(x^2) + eps) for x in [128, 16] column layout."""
                sq = small.tile([128, 16], dt, tag=f"sq_{name}")
                ssum = small.tile([128, 1], dt, tag=f"ss_{name}")
                nc.scalar.activation(sq[:], xt[:], AF.Square, accum_out=ssum[:])
                tot = ps.tile([1, 1], dt, tag="b")
                nc.tensor.matmul(tot[:], ssum[:], ones_col[:], start=True, stop=True)
                std = small.tile([1, 1], dt, tag=f"std_{name}")
                nc.scalar.activation(
                    std[:], tot[:], AF.Sqrt, bias=eps_c[:], scale=1.0 / float(E)
                )
                inv = small.tile([1, 1], dt, tag=f"inv_{name}")
                nc.vector.reciprocal(inv[:], std[:])
                invb_ps = ps.tile([128, 1], dt, tag="b")
                nc.tensor.matmul(
                    invb_ps[:], ones_row[:], inv[:], start=True, stop=True
                )
                xn = small.tile([128, 16], bf if out_bf else dt, tag=f"xn_{name}")
                nc.vector.tensor_scalar_mul(xn[:], xt[:], invb_ps[:])
                return xn

            def to_bf(xt, name, w=16):
                xb = small.tile([128, w], bf, tag=f"bf_{name}")
                nc.vector.tensor_copy(xb[:], xt[:])
                return xb

            def row_to_col(row_sb, nblk, name):
                """[1, nblk*128] row (on one partition) -> PSUM [128, nblk]
                columns via nblk K=1 matmuls (no DMA round-trip)."""
                out = ps.tile([128, nblk], dt, tag="b", name=f"r2c_{name}")
                for c in range(nblk):
                    nc.tensor.matmul(
                        out[:, c : c + 1],
                        row_sb[0:1, c * 128 : (c + 1) * 128],
                        one1[:],
                        start=True, stop=True,
                    )
                return out

            # ---- embedding + double rms ----
            xw = stage.tile([128, 16], dt, tag="xw")
            nc.scalar.dma_start(xw[:], xe_wte[:])
            xp = stage.tile([128, 16], dt, tag="xp")
            nc.scalar.dma_start(xp[:], xe_wpe[:])
            x0 = small.tile([128, 16], dt, tag="x0")
            nc.vector.tensor_add(x0[:], xw[:], xp[:])
            x1 = rms(x0, "n1")  # residual input
            x2b = rms(x1, "n2", out_bf=True)

            # ---- qkv projection: [1, 768] row (q 0:256 | k 256:512 | v 512:768)
            # SCALE is folded into wq on the host. Two col-tiled chains
            # (rows 0 and 32 of one PSUM bank) run concurrently on PE.
            qkv_ps = ps.tile([128, 512], dt, tag="b", name="qkv_ps")
            for t in range(4):
                wt = stile("qkv", 3072)
                nc.sync.dma_start(wt[:], wqkv_r[:, t * 3072 : (t + 1) * 3072])
                for b in range(4):
                    k = 4 * t + b
                    nc.tensor.matmul(
                        qkv_ps[0:1, 0:512], x2b[:, k : k + 1],
                        wt[:, b * 768 : b * 768 + 512],
                        start=(k == 0), stop=(k == 15),
                    )
                    nc.tensor.matmul(
                        qkv_ps[32:33, 0:256], x2b[:, k : k + 1],
                        wt[:, b * 768 + 512 : b * 768 + 768],
                        start=(k == 0), stop=(k == 15),
                        tile_position=(0, 32),
                    )
            qkv_row = small.tile([1, 768], dt, tag="qkv")
            nc.scalar.copy(qkv_row[:, 0:512], qkv_ps[0:1, 0:512])
            nc.scalar.copy(qkv_row[:, 512:768], qkv_ps[32:33, 0:256])

            # ---- q,k to column layout via K=1 matmuls ----
            qkT_ps = row_to_col(qkv_row, 4, "qk")
            qkTb = small.tile([128, 4], bf, tag="qkTb")
            nc.vector.tensor_copy(qkTb[:], qkT_ps[:])

            # ---- attention scores in column layout: per head h and 128-row
            # t-block c, matmul(out[128,1], lhsT=K_block[128d,128t], rhs=q[128,1])
            # gives scores for t in [c*128, (c+1)*128) on partitions. ACT exp
            # straight from PSUM, accum_out accumulates softmax denominators.
            wTs = []
            esp = small.tile([128, 4], dt, tag="esp")  # per-partition exp sums
            for h in range(HPC):
                wTs.append(small.tile([128, 64], bf, tag=f"wT{h}", name=f"wT{h}"))
            for h in range(HPC):
                for j in range(2):
                    kt = stile("key", 4096)
                    nc.sync.dma_start(
                        kt[:],
                        keys_r[:, h * 8192 + j * 4096 : h * 8192 + (j + 1) * 4096],
                    )
                    qk_ps = ps.tile([128, 32], dt, tag="b", name=f"qk{h}_{j}")
                    for c in range(32):
                        nc.tensor.matmul(
                            qk_ps[:, c : c + 1],
                            kt[:, c * 128 : (c + 1) * 128],
                            qkTb[:, h : h + 1],
                            start=True, stop=True,
                        )
                    nc.scalar.activation(
                        wTs[h][:, j * 32 : (j + 1) * 32], qk_ps[:], AF.Exp,
                        accum_out=esp[:, h * 2 + j : h * 2 + j + 1],
                    )

            # current-token score per head: exp(q_h . k_h) (SCALE folded into q)
            e_last = small.tile([1, 2], dt, tag="elast")
            for h in range(HPC):
                pal = ps.tile([1, 1], dt, tag="b")
                nc.tensor.matmul(
                    pal[:], qkTb[:, h : h + 1], qkTb[:, 2 + h : 3 + h],
                    start=True, stop=True,
                )
                nc.scalar.activation(e_last[:, h : h + 1], pal[:], AF.Exp)

            # softmax denominators: cross-partition sum of esp + e_last
            dps = ps.tile([1, 4], dt, tag="b")
            nc.tensor.matmul(dps[:], ones_col[:], esp[:], start=True, stop=True)
            dtmp = small.tile([1, 2], dt, tag="dtmp")
            for h in range(HPC):
                nc.vector.reduce_sum(
                    dtmp[:, h : h + 1], dps[:, h * 2 : (h + 1) * 2],
                    axis=mybir.AxisListType.X,
                )
            nc.vector.tensor_add(dtmp[:], dtmp[:], e_last[:])
            dinv = small.tile([1, 2], dt, tag="dinv")
            nc.vector.reciprocal(dinv[:], dtmp[:])

            # ---- PV on PE: x_attn_h = sum_t w[t] * V[t, :] (unnormalized).
            # Per t-block c: matmul(out[1,128], lhsT=wT[:, c], rhs=V_block),
            # accumulating over all 64 blocks; the two heads run in different
            # PE column groups (rows 0 / 32 of one bank).
            pv_ps = ps.tile([128, 128], dt, tag="b", name="pv_ps")
            for tt in range(4):
                vt = stile("val", 4096)
                nc.sync.dma_start(vt[:], vals_r[:, tt * 4096 : (tt + 1) * 4096])
                for b in range(16):
                    c = tt * 16 + b
                    for h in range(HPC):
                        nc.tensor.matmul(
                            pv_ps[32 * h : 32 * h + 1, :],
                            wTs[h][:, c : c + 1],
                            vt[:, b * 256 + h * 128 : b * 256 + (h + 1) * 128],
                            start=(c == 0), stop=(c == 63),
                            tile_position=(0, 32 * h),
                        )

            # combine with current-token value, then normalize by the softmax sum
            xa_row = small.tile([1, 256], dt, tag="xa")
            for h in range(HPC):
                sl = slice(h * 128, (h + 1) * 128)
                nc.vector.tensor_scalar_mul(
                    xa_row[:, sl],
                    qkv_row[:, 512 + h * 128 : 512 + (h + 1) * 128],
                    e_last[:, h : h + 1],
                )
                nc.vector.tensor_add(
                    xa_row[:, sl], xa_row[:, sl], pv_ps[32 * h : 32 * h + 1, :]
                )
                nc.vector.tensor_scalar_mul(xa_row[:, sl], xa_row[:, sl], dinv[:, h : h + 1])

            xaT_ps = row_to_col(xa_row, 2, "xa")
            xaTb = small.tile([128, 2], bf, tag="xaTb")
            nc.vector.tensor_copy(xaTb[:], xaT_ps[:])

            # ---- wo partial: [1, 2048] via 4 col-tiled PE chains ----
            ar1_in = small.tile([1, 2048], dt, tag="arin", name="ar1_in")
            wo_ps = ps.tile([128, 512], dt, tag="b", name="wo_ps")
            wot = stile("wo", 4096)
            nc.sync.dma_start(wot[:], wo_r[:])
            for k in range(2):
                for n in range(4):
                    nc.tensor.matmul(
                        wo_ps[32 * n : 32 * n + 1, :], xaTb[:, k : k + 1],
                        wot[:, k * 2048 + n * 512 : k * 2048 + (n + 1) * 512],
                        start=(k == 0), stop=(k == 1),
                        tile_position=(0, 32 * n),
                    )
            ar1_d = dram.tile([1, 2048], dt, tag="ar1_in_d")
            for n in range(4):
                sl = slice(n * 512, (n + 1) * 512)
                eng = nc.vector.tensor_copy if n % 2 == 0 else nc.scalar.copy
                eng(ar1_in[:, sl], wo_ps[32 * n : 32 * n + 1, :])
                nc.scalar.dma_start(ar1_d[:, sl], ar1_in[:, sl])

            def all_reduce(in_d, name):
                """Sum a [1, 2048] DRAM partial across cores; returns PSUM
                [128, 16]. CCE AllReduce in DRAM, flat readback, then 16 K=1
                matmuls rebuild the column layout."""
                out_d = dram.tile([1, 2048], dt, tag=f"{name}_out")
                nc.gpsimd.collective_compute(
                    "AllReduce",
                    ADD,
                    replica_groups=[list(range(N_CORES))],
                    ins=[in_d.opt()],
                    outs=[out_d.opt()],
                )
                rb = stage.tile([1, 2048], dt, tag="rb", name=f"rb_{name}", bufs=1)
                nc.scalar.dma_start(rb[:], out_d[:])
                return row_to_col(rb, 16, name)

            x3_ps = all_reduce(ar1_d, "ar1")
            # re-preload the Sqrt LUT while the AllReduce is in flight
            nc.scalar.sqrt(junk[:], eps_c[:])
            x3 = small.tile([128, 16], dt, tag="x3")
            nc.vector.tensor_add(x3[:], x3_ps[:], x1[:])  # + residual

            # ---- MLP1: h = relu(w1 @ x4) via 2 col-tiled PE chains ----
            x4b = rms(x3, "n3", out_bf=True)
            mh_ps = ps.tile([128, 512], dt, tag="b", name="mh_ps")
            for a in range(4):
                w1t = stile("w1", 4096)
                nc.sync.dma_start(w1t[:], w1_r[:, a * 4096 : (a + 1) * 4096])
                for b in range(4):
                    k = a * 4 + b
                    for n in range(2):
                        nc.tensor.matmul(
                            mh_ps[32 * n : 32 * n + 1, :], x4b[:, k : k + 1],
                            w1t[:, b * 1024 + n * 512 : b * 1024 + (n + 1) * 512],
                            start=(k == 0), stop=(k == 15),
                            tile_position=(0, 32 * n),
                        )
            h_row = small.tile([1, 1024], dt, tag="hrow")
            for n in range(2):
                nc.scalar.activation(
                    h_row[:, n * 512 : (n + 1) * 512],
                    mh_ps[32 * n : 32 * n + 1, :], AF.Relu,
                )

            hT_ps = row_to_col(h_row, 8, "h")
            hTb = small.tile([128, 8], bf, tag="hTb")
            nc.vector.tensor_copy(hTb[:], hT_ps[:])

            # ---- MLP2: [1, 2048] partial via 4 col-tiled PE chains ----
            ar2_in = small.tile([1, 2048], dt, tag="arin", name="ar2_in")
            m2_ps = ps.tile([128, 512], dt, tag="b", name="m2_ps")
            for a in range(4):
                w2t = stile("w2", 4096)
                nc.sync.dma_start(w2t[:], w2_r[:, a * 4096 : (a + 1) * 4096])
                for b in range(2):
                    k = a * 2 + b
                    for n in range(4):
                        nc.tensor.matmul(
                            m2_ps[32 * n : 32 * n + 1, :], hTb[:, k : k + 1],
                            w2t[:, b * 2048 + n * 512 : b * 2048 + (n + 1) * 512],
                            start=(k == 0), stop=(k == 7),
                            tile_position=(0, 32 * n),
                        )
            ar2_d = dram.tile([1, 2048], dt, tag="ar2_in_d")
            for n in range(4):
                sl = slice(n * 512, (n + 1) * 512)
                eng = nc.vector.tensor_copy if n % 2 == 0 else nc.scalar.copy
                eng(ar2_in[:, sl], m2_ps[32 * n : 32 * n + 1, :])
                nc.scalar.dma_start(ar2_d[:, sl], ar2_in[:, sl])

            x5_ps = all_reduce(ar2_d, "ar2")
            x5b = small.tile([128, 16], bf, tag="x5b")
            nc.vector.tensor_add(x5b[:], x5_ps[:], x3[:])  # + residual, cast bf16

            # ---- LM head over the vocab shard: a single k-loop of full-width
            # [128, 6283] tiles. 13 output chunks as col-tiled PE chains, 4 per
            # PSUM bank (partition rows 0/32/64/96), all accumulating over the
            # 16 k-blocks concurrently.
            NCH = (VPC + 511) // 512  # 13
            banks = [
                ps.tile([128, 512], dt, tag="lmb", name=f"lmb{b}", bufs=4)
                for b in range((NCH + 3) // 4)
            ]

            def chain_out(c, cw):
                return banks[c // 4][32 * (c % 4) : 32 * (c % 4) + 1, 0:cw]

            for k in range(16):
                lt = stile("lm", VPC)
                nc.sync.dma_start(lt[:], lm_r[:, k * VPC : (k + 1) * VPC])
                for c in range(NCH):
                    cw = min(512, VPC - c * 512)
                    nc.tensor.matmul(
                        chain_out(c, cw), x5b[:, k : k + 1],
                        lt[:, c * 512 : c * 512 + cw],
                        start=(k == 0), stop=(k == 15),
                        tile_position=(0, 32 * (c % 4)),
                    )
            for b in range((NCH + 3) // 4):
                nch_b = min(4, NCH - 4 * b)  # chunks in this bank
                wid = VPC - 4 * b * 512 if nch_b < 4 else 2048
                ldr = small.tile([128, 512], dt, tag="ldr", name=f"ldr{b}", bufs=2)
                eng = nc.vector.tensor_copy if b % 2 == 0 else nc.scalar.copy
                eng(ldr[:, :], banks[b][:, :])
                if b == 0:
                    # keep the warmup-collective result alive off the
                    # critical path: ldr[0, :16] += 0 * warm
                    nc.vector.scalar_tensor_tensor(
                        ldr[0:1, 0:16], warm_back[:], 0.0, ldr[0:1, 0:16],
                        op0=MUL, op1=ADD,
                    )
                if nch_b == 4:
                    nc.scalar.dma_start(
                        logits_out[:, b * 2048 : b * 2048 + 2048],
                        ldr[0:128:32, :],
                    )
                else:
                    for r in range(nch_b):
                        c = 4 * b + r
                        cw = min(512, VPC - c * 512)
                        nc.scalar.dma_start(
                            logits_out[:, c * 512 : c * 512 + cw],
                            ldr[32 * r : 32 * r + 1, 0:cw],
                        )

    nc.finalize()
    return nc


def _col16(v):
    """[2048] vector -> [128, 16] column-major layout (e = c*128 + p at [p, c])."""
    return np.ascontiguousarray(v.reshape(16, 128).T)


def _part_major(mT, nblk, blk_rows, width):
    """[nblk*blk_rows, width] -> [blk_rows, nblk*width] partition-major, bf16."""
    out = mT.reshape(nblk, blk_rows, width).transpose(1, 0, 2).reshape(
        blk_rows, nblk * width
    )
    return np.ascontiguousarray(out.astype(ml_dtypes.bfloat16))


def _prep_in_maps(token_id, pos_id, keys, values, wte, wpe, wq, wk, wv, wo, w1, w2, lm_w):
    f32 = lambda a: np.asarray(a, dtype=np.float32)
    keys, values = f32(keys), f32(values)
    wq, wk, wv, wo, w1, w2, lm_w = map(f32, (wq, wk, wv, wo, w1, w2, lm_w))
    wq = wq * np.float32(SCALE)  # fold attention scale into q
    xe_wte = _col16(f32(wte[token_id]))
    xe_wpe = _col16(f32(wpe[pos_id]))
    lm_pad = np.zeros((N_CORES * VPC, E), np.float32)
    lm_pad[:VOCAB] = lm_w

    in_maps = []
    for i in range(N_CORES):
        hs = slice(i * EPC, (i + 1) * EPC)
        wqkv = np.concatenate([wq[hs], wk[hs], wv[hs]], axis=0)  # [768, E]
        in_maps.append(
            {
                "xe_wte": xe_wte,
                "xe_wpe": xe_wpe,
                "wqkv_r": _part_major(np.ascontiguousarray(wqkv.T), 16, 128, 768),
                "keys_r": _part_major(np.ascontiguousarray(keys[:, hs].T), 2, 128, 8192),
                "vals_r": _part_major(values[:, hs], 64, 128, EPC),
                "wo_r": _part_major(np.ascontiguousarray(wo[:, hs].T), 2, 128, E),
                "w1_r": _part_major(
                    np.ascontiguousarray(w1[i * 1024 : (i + 1) * 1024].T), 16, 128, 1024
                ),
                "w2_r": _part_major(
                    np.ascontiguousarray(w2[:, i * 1024 : (i + 1) * 1024].T), 8, 128, E
                ),
                "lm_r": _part_major(
                    np.ascontiguousarray(lm_pad[i * VPC : (i + 1) * VPC].T), 16, 128, VPC
                ),
            }
        )
    return in_maps


def kernel(**inputs) -> np.ndarray:
    from concourse.bass_utils import run_bass_kernel_spmd

    token_id = int(inputs["token_id"])
    pos_id = int(inputs["pos_id"])
    in_maps = _prep_in_maps(
        token_id,
        pos_id,
        inputs["keys"],
        inputs["values"],
        inputs["wte"],
        inputs["wpe"],
        inputs["wq"],
        inputs["wk"],
        inputs["wv"],
        inputs["wo"],
        inputs["w1"],
        inputs["w2"],
        inputs["lm_w"],
    )
    if "nc" not in _CACHE:
        _CACHE["nc"] = _build_nc()
    nc = _CACHE["nc"]
    res = run_bass_kernel_spmd(
        nc,
        in_maps,
        core_ids=list(range(N_CORES)),
        trace=TRACE,
        trace_cores=[0] if TRACE else None,
    )
    _CACHE["last_result"] = res
    logits = np.concatenate([r["logits"][0] for r in res.results])[:VOCAB]
    return np.ascontiguousarray(logits.astype(np.float32))
